# revision 3
# baseline (speedup 1.0000x reference)
"""Causal self-attention with RoPE on 8 Trainium2 NeuronCores.

Sharding: batch x head-group. Core c handles batch b = c//2 and head group
g = c%2 (8 of 16 heads). Each core runs the full per-(batch, head-group)
pipeline on device:

  QKV^T projection -> RoPE -> causal flash-style attention -> partial
  output projection (its heads' slice of W_out rows).

The host sums the two partial projections per batch and adds b_out.

Device layout choices (all matmuls contract over the partition dim):
  - x is fed pre-transposed (xT: [D, L]) so Q^T/K^T = W^T x^T come out with
    head dims on partitions, which is exactly the lhsT/rhs layout the
    score matmul S^T = K Q^T wants.  V is computed in natural [L, dv]
    layout (lhsT = xT tile), which is the lhsT layout the PV matmul wants.
  - S^T = matmul(lhsT=K^T tile, rhs=Q^T tile) comes out [lk, lq]; exp(S^T)
    is then directly the lhsT-side...  actually rhs of the PV matmul:
    Y^T = matmul(lhsT=V_aug, rhs=expS^T).  A ones column appended to V
    yields the softmax denominator for free in row 64 of the PV psum.
  - Softmax uses no max subtraction: scores are O(1) here (|s|/sqrt(dh)
    stays far below fp32/bf16 exp range), so exp/sum/divide is exact.
  - Causal masking is multiplicative on exp(S^T) (0/1 mask slices), only
    needed on the 4 diagonal 128-tiles of each 512-wide query chunk.
"""

import os
import sys

if "/opt/trn_rl_repo" not in sys.path:
    sys.path.insert(0, "/opt/trn_rl_repo")

import numpy as np
import ml_dtypes

import concourse.bass as bass
import concourse.mybir as mybir
import concourse.tile as tile

F32 = mybir.dt.float32
F32R = mybir.dt.float32r
BF16 = mybir.dt.bfloat16

B, L, D = 4, 2048, 1024
H, DH = 16, 64
NCORES = 8
G = 2                 # head groups (cores per batch)
HPC = H // G          # heads per core = 8
DQ = HPC * DH         # per-core q/k/v width = 512
PAIRS = HPC // 2      # 128-partition head pairs = 4
CHUNK = 512           # query-chunk (matmul free dim)
NCH = L // CHUNK      # 4
KT = D // 128         # 8 k-tiles over d_model
LT = L // 128         # 16 l-tiles
VW = DH + 1           # V columns per head incl. ones column = 65

LAST_RESULTS = None   # test harness reads perf fields from here


def legalize_bir_waits(bir_json: bytes) -> bytes:
    """Split multi-wait sync_infos into standalone EventSemaphore instrs.

    This container's walrus codegen accepts at most ONE sync wait per
    instruction (two for EventSemaphore), but Tile's sem assigner happily
    attaches several.  For every instruction carrying N>1 waits, keep one
    and hoist the rest onto EventSemaphore instructions inserted directly
    before it on the same engine (same block), which preserves each
    engine's program order and therefore the sync semantics.
    """
    import json as _json

    j = _json.loads(bir_json)
    uid = [0]
    for fn in j["functions"]:
        for blk in fn["blocks"]:
            out_insts = []
            for inst in blk["instructions"]:
                si = inst.get("sync_info")
                waits = (si or {}).get("on_wait") or []
                cap = 2 if inst.get("opcode") == "EventSemaphore" else 1
                if len(waits) > cap:
                    extra, keep = waits[:-cap], waits[-cap:]
                    for i in range(0, len(extra), 2):
                        uid[0] += 1
                        out_insts.append(
                            {
                                "name": f"antwaitfix-{uid[0]}",
                                "opcode": "EventSemaphore",
                                "engine": inst["engine"],
                                "ins": [],
                                "outs": [],
                                "debug": inst.get("debug", 0),
                                "sync_info": {
                                    "on_wait": extra[i : i + 2],
                                    "on_update": [],
                                },
                            }
                        )
                    si["on_wait"] = keep
                out_insts.append(inst)
            blk["instructions"] = out_insts
    return _json.dumps(j).encode()


def build_module():
    nc = bass.Bass(use_seq_codegen=True)

    xT = nc.declare_dram_parameter("xT", [D, L], BF16, isOutput=False)
    wq = nc.declare_dram_parameter("wq", [D, DQ], BF16, isOutput=False)
    wk = nc.declare_dram_parameter("wk", [D, DQ], BF16, isOutput=False)
    wv = nc.declare_dram_parameter("wv", [D, DQ], BF16, isOutput=False)
    wo = nc.declare_dram_parameter("wo", [DQ, D], BF16, isOutput=False)
    bq = nc.declare_dram_parameter("bq", [128, PAIRS], F32, isOutput=False)
    bk = nc.declare_dram_parameter("bk", [128, PAIRS], F32, isOutput=False)
    bv = nc.declare_dram_parameter("bv", [128, DQ], F32, isOutput=False)
    cosT = nc.declare_dram_parameter("cosT", [128, L], BF16, isOutput=False)
    sinT = nc.declare_dram_parameter("sinT", [128, L], BF16, isOutput=False)
    maskb = nc.declare_dram_parameter("maskb", [128, 896], BF16, isOutput=False)
    out = nc.declare_dram_parameter("out", [L, D], F32, isOutput=True)

    with tile.TileContext(nc) as tc:
        with (
            tc.tile_pool(name="const", bufs=1) as cp,
            tc.tile_pool(name="acts", bufs=1) as ap,
            tc.tile_pool(name="work", bufs=5) as wp,
            tc.tile_pool(name="pss", bufs=4, space="PSUM") as pss,
            tc.tile_pool(name="psy", bufs=4, space="PSUM") as psy,
        ):
            # ---- constant / activation loads (split for DMA-queue spread)
            xT_sb = ap.tile([128, KT, L], BF16)
            for kt in range(KT):
                nc.sync.dma_start(
                    xT_sb[:, kt, :],
                    xT.rearrange("(kt p) l -> p kt l", p=128)[:, kt, :],
                )
            wq_sb = cp.tile([128, KT, DQ], BF16)
            wk_sb = cp.tile([128, KT, DQ], BF16)
            wv_sb = cp.tile([128, KT, DQ], BF16)
            for kt in range(KT):
                nc.sync.dma_start(
                    wq_sb[:, kt, :], wq.rearrange("(kt p) m -> p kt m", p=128)[:, kt, :]
                )
                nc.sync.dma_start(
                    wk_sb[:, kt, :], wk.rearrange("(kt p) m -> p kt m", p=128)[:, kt, :]
                )
                nc.sync.dma_start(
                    wv_sb[:, kt, :], wv.rearrange("(kt p) m -> p kt m", p=128)[:, kt, :]
                )
            wo_sb = cp.tile([128, PAIRS, D], BF16)
            for pr in range(PAIRS):
                nc.sync.dma_start(
                    wo_sb[:, pr, :], wo.rearrange("(pr p) c -> p pr c", p=128)[:, pr, :]
                )
            bq_sb = cp.tile([128, PAIRS], F32)
            bk_sb = cp.tile([128, PAIRS], F32)
            bv_sb = cp.tile([128, DQ], F32)
            cos_sb = cp.tile([128, L], BF16)
            sin_sb = cp.tile([128, L], BF16)
            mask_sb = cp.tile([128, 896], BF16)
            nc.sync.dma_start(bq_sb[:], bq[:])
            nc.sync.dma_start(bk_sb[:], bk[:])
            nc.sync.dma_start(bv_sb[:], bv[:])
            nc.sync.dma_start(cos_sb[:], cosT[:])
            nc.sync.dma_start(sin_sb[:], sinT[:])
            nc.sync.dma_start(mask_sb[:], maskb[:])
            # memset can't encode a float32r immediate; memset f32 then
            # copy-convert (bitwise identical) into the f32r tile.
            ones_f32 = cp.tile([128, 64], F32)
            nc.vector.memset(ones_f32[:], 1.0)
            ones_sb = cp.tile([128, 64], F32R)
            with nc.allow_low_precision(reason="f32r ones for bcast mm"):
                nc.vector.tensor_copy(ones_sb[:], ones_f32[:])

            qT_sb = ap.tile([128, PAIRS, L], BF16)
            kT_sb = ap.tile([128, PAIRS, L], BF16)
            v_sb = ap.tile([128, LT, HPC * VW], BF16)
            yT_sb = ap.tile([128, PAIRS, L], BF16)

            # ---- phase 1: QKV projection
            for mt in range(PAIRS):
                for c in range(NCH):
                    for dst, w_sb, b_sb in ((qT_sb, wq_sb, bq_sb), (kT_sb, wk_sb, bk_sb)):
                        ps = pss.tile([128, CHUNK], F32, tag="ps")
                        for kt in range(KT):
                            nc.tensor.matmul(
                                ps[:],
                                w_sb[:, kt, mt * 128 : (mt + 1) * 128],
                                xT_sb[:, kt, c * CHUNK : (c + 1) * CHUNK],
                                start=(kt == 0),
                                stop=(kt == KT - 1),
                            )
                        nc.scalar.activation(
                            dst[:, mt, c * CHUNK : (c + 1) * CHUNK],
                            ps[:],
                            mybir.ActivationFunctionType.Identity,
                            bias=b_sb[:, mt : mt + 1],
                        )
            for lt in range(LT):
                ps = pss.tile([128, CHUNK], F32, tag="ps")
                for kt in range(KT):
                    nc.tensor.matmul(
                        ps[:],
                        xT_sb[:, kt, lt * 128 : (lt + 1) * 128],
                        wv_sb[:, kt, :],
                        start=(kt == 0),
                        stop=(kt == KT - 1),
                    )
                vdst = v_sb[:, lt, :].rearrange("p (h c) -> p h c", c=VW)
                nc.vector.tensor_add(vdst[:, :, 0:DH], ps[:], bv_sb[:])
                nc.vector.memset(vdst[:, :, DH:VW], 1.0)

            # ---- phase 1.5: RoPE on Q^T and K^T (in place)
            for dst in (qT_sb, kT_sb):
                for mt in range(PAIRS):
                    t = dst[:, mt, :]
                    swp = wp.tile([128, L], BF16, tag="swp")
                    for i in range(4):
                        j = i ^ 1
                        nc.sync.dma_start(
                            swp[i * 32 : (i + 1) * 32, :], t[j * 32 : (j + 1) * 32, :]
                        )
                    nc.vector.tensor_mul(swp[:], swp[:], sin_sb[:])
                    nc.vector.tensor_mul(t, t, cos_sb[:])
                    nc.vector.tensor_add(t, t, swp[:])

            # ---- phase 2: attention (chunk-outer; the two pairs of each
            # half interleave at the kt level so PE always has independent
            # score/PV work while ACT runs the other pair's exp)
            for c in range(NCH):
                q0 = c * CHUNK
                n_lk = (q0 + CHUNK) // 128
                for half in range(2):
                    prs = (2 * half, 2 * half + 1)
                    ys = {
                        pr: [
                            psy.tile(
                                [128, CHUNK], F32, tag="psy",
                                name=f"psy_{pr}_{c}_{i}",
                            )
                            for i in range(2)
                        ]
                        for pr in prs
                    }
                    for kt in range(n_lk):
                        k0 = kt * 128
                        for pr in prs:
                            kT_p = kT_sb[:, pr, :]
                            qT_p = qT_sb[:, pr, :]
                            exps = []
                            for hh in range(2):
                                ps = pss.tile([128, CHUNK], F32, tag="ps")
                                nc.tensor.matmul(
                                    ps[:],
                                    kT_p[hh * 64 : (hh + 1) * 64, k0 : k0 + 128],
                                    qT_p[hh * 64 : (hh + 1) * 64, q0 : q0 + CHUNK],
                                    start=True,
                                    stop=True,
                                )
                                ex = wp.tile([128, CHUNK], BF16, tag="exp")
                                nc.scalar.activation(
                                    ex[:], ps[:], mybir.ActivationFunctionType.Exp,
                                    scale=float(1.0 / np.sqrt(DH)),
                                )
                                if k0 >= q0:
                                    s = 384 - (k0 - q0)
                                    nc.vector.tensor_mul(
                                        ex[:], ex[:], mask_sb[:, s : s + CHUNK]
                                    )
                                exps.append(ex)
                            for hh in range(2):
                                h = 2 * pr + hh
                                nc.tensor.matmul(
                                    ys[pr][hh][0:VW, :],
                                    v_sb[:, kt, h * VW : (h + 1) * VW],
                                    exps[hh][:],
                                    start=(kt == 0),
                                    stop=(kt == n_lk - 1),
                                )
                    for pr in prs:
                        for hh in range(2):
                            den = wp.tile([128, CHUNK], F32R, tag="den")
                            with nc.allow_low_precision(reason="f32r recip"):
                                nc.vector.reciprocal(
                                    den[64:65, :], ys[pr][hh][64:65, :]
                                )
                            bc = pss.tile([128, CHUNK], F32, tag="ps")
                            nc.tensor.matmul(
                                bc[0:64, :],
                                ones_sb[64:65, :],
                                den[64:65, :],
                                start=True,
                                stop=True,
                            )
                            # DVE has a single PSUM port: stage the broadcast
                            # through SBUF (ScalarE copy) so the multiply
                            # reads only one PSUM operand.
                            bcs = wp.tile([64, CHUNK], F32, tag="bcs")
                            nc.scalar.copy(bcs[:], bc[0:64, :])
                            if hh == 0:
                                nc.vector.tensor_mul(
                                    yT_sb[0:64, pr, q0 : q0 + CHUNK],
                                    ys[pr][hh][0:64, :],
                                    bcs[:],
                                )
                            else:
                                # walrus rejects elementwise ops whose out/in
                                # partition bases differ; base-0 temp + DMA
                                # does the partition move.
                                yt = wp.tile([64, CHUNK], BF16, tag="ytmp")
                                nc.vector.tensor_mul(
                                    yt[:], ys[pr][hh][0:64, :], bcs[:]
                                )
                                nc.sync.dma_start(
                                    yT_sb[64:128, pr, q0 : q0 + CHUNK], yt[:]
                                )

                # ---- phase 3 (interleaved): output projection for this
                # chunk's l-tiles, partial over this core's W_out rows
                for lt in range(4 * c, 4 * c + 4):
                    for cc in range(2):
                        ps = pss.tile([128, CHUNK], F32, tag="ps")
                        for pr in range(PAIRS):
                            nc.tensor.matmul(
                                ps[:],
                                yT_sb[:, pr, lt * 128 : (lt + 1) * 128],
                                wo_sb[:, pr, cc * CHUNK : (cc + 1) * CHUNK],
                                start=(pr == 0),
                                stop=(pr == PAIRS - 1),
                            )
                        ob = wp.tile([128, CHUNK], F32, tag="ob")
                        nc.vector.tensor_copy(ob[:], ps[:])
                        nc.sync.dma_start(
                            out[
                                lt * 128 : (lt + 1) * 128,
                                cc * CHUNK : (cc + 1) * CHUNK,
                            ],
                            ob[:],
                        )
    return nc


def _rope_tables():
    inv_freq = (1.0 / (10000.0 ** (np.arange(0, DH, 2, dtype=np.float32) / DH))).astype(
        np.float32
    )
    t = np.arange(L, dtype=np.float32)
    freqs = np.einsum("l,d->ld", t, inv_freq).astype(np.float32)  # (L, 32)
    emb = np.concatenate([freqs, freqs], axis=-1)                 # (L, 64)
    cos = np.cos(emb).astype(np.float32)
    sin = np.sin(emb).astype(np.float32)
    cosT = cos.T                                   # (64, L)
    sinT = sin.T.copy()
    sinT[0:32] = -sinT[0:32]                       # fold rotate_half sign
    cos128 = np.tile(cosT, (2, 1))                 # (128, L)
    sin128 = np.tile(sinT, (2, 1))
    return cos128, sin128


def _mask_big():
    # maskb[p, j] = 1.0 iff p <= j - 384 (slice at s = 384-delta gives the
    # diagonal-tile mask "p <= f - delta")
    p = np.arange(128)[:, None]
    j = np.arange(896)[None, :]
    return (p <= j - 384).astype(np.float32)


def _bf16(a):
    return np.asarray(a, dtype=np.float32).astype(ml_dtypes.bfloat16)


_COMPILED = None


def _ensure_trace_hook() -> bool:
    """Install the axon NTFF profile hook if the boot shim couldn't.

    The image's `antenv` stub lacks `axon_hooks`, so bass_utils' trace
    path crashes on import. Synthesize the module and wire in the ctypes
    hook from trn_agent_boot. Returns True iff tracing is usable.
    """
    try:
        from antenv.axon_hooks import get_axon_ntff_profile_hook  # noqa: F401

        return True
    except ImportError:
        pass
    try:
        import types

        import antenv
        import trn_agent_boot.trn_boot as tb

        mod = types.ModuleType("antenv.axon_hooks")
        _hook = [None]
        mod.set_axon_ntff_profile_hook = lambda h: _hook.__setitem__(0, h)
        mod.get_axon_ntff_profile_hook = lambda: _hook[0]
        sys.modules["antenv.axon_hooks"] = mod
        antenv.axon_hooks = mod
        mod.set_axon_ntff_profile_hook(
            tb._ntff_profile_via_ctypes("/opt/axon/libaxon_pjrt.so")
        )
        return True
    except Exception:
        return False


def kernel(x, pad_mask, W_qkv, b_qkv, W_out, b_out):
    global LAST_RESULTS, _COMPILED
    from concourse.bass_utils import run_bass_kernel_spmd

    x = np.asarray(x, dtype=np.float32)
    W_qkv = np.asarray(W_qkv, dtype=np.float32)
    b_qkv = np.asarray(b_qkv, dtype=np.float32)
    W_out = np.asarray(W_out, dtype=np.float32)
    b_out = np.asarray(b_out, dtype=np.float32)

    cos128, sin128 = _rope_tables()
    maskb = _mask_big()

    in_maps = []
    for core in range(NCORES):
        b, g = core // G, core % G
        sl = slice(g * DQ, (g + 1) * DQ)
        wq = W_qkv[:, 0 * D : 1 * D][:, sl]
        wk = W_qkv[:, 1 * D : 2 * D][:, sl]
        wv = W_qkv[:, 2 * D : 3 * D][:, sl]
        bqv = b_qkv[0 * D : 1 * D][sl]
        bkv = b_qkv[1 * D : 2 * D][sl]
        bvv = b_qkv[2 * D : 3 * D][sl]
        in_maps.append(
            {
                "xT": _bf16(x[b].T),
                "wq": _bf16(wq),
                "wk": _bf16(wk),
                "wv": _bf16(wv),
                "wo": _bf16(W_out[sl, :]),
                "bq": np.ascontiguousarray(bqv.reshape(PAIRS, 128).T),
                "bk": np.ascontiguousarray(bkv.reshape(PAIRS, 128).T),
                "bv": np.tile(bvv[None, :], (128, 1)).astype(np.float32),
                "cosT": _bf16(cos128),
                "sinT": _bf16(sin128),
                "maskb": _bf16(maskb),
            }
        )

    if _COMPILED is None:
        nc = build_module()
        fixed = legalize_bir_waits(nc.to_json_bytes())
        nc.to_json_bytes = lambda: fixed  # bass2jax ships this BIR to walrus
        _COMPILED = nc
    nc = _COMPILED

    res = run_bass_kernel_spmd(
        nc,
        in_maps,
        core_ids=list(range(NCORES)),
        trace=bool(os.environ.get("BASS_TRACE")) and _ensure_trace_hook(),
    )
    LAST_RESULTS = res

    out = np.zeros((B, L, D), dtype=np.float32)
    for core in range(NCORES):
        out[core // G] += np.asarray(res.results[core]["out"], dtype=np.float32)
    out += b_out[None, None, :]
    return out



# revision 13
# speedup vs baseline: 1.2883x; 1.2883x over previous
"""Causal self-attention with RoPE on 8 Trainium2 NeuronCores.

Sharding: batch x head-group. Core c handles batch b = c//2 and head group
g = c%2 (8 of 16 heads). Each core runs the full per-(batch, head-group)
pipeline on device; the host sums the two partial output projections per
batch and adds b_out.

v2 layout (chunk-pipelined for PE warmth):
  The TRN2 PE clock-gates to 1.2 GHz after any ~3.4us idle window and only
  reaches 2.4 GHz under sustained work, so the whole kernel is emitted as a
  single software-pipelined stream: QKV projection for chunk c+1 and the
  output projection for chunk c are "filler" PE work that the Tile list
  scheduler pulls into the gaps of chunk c's attention (which is paced by
  ACT exp). PSUM budget (8 banks): 2 x [128,1024] score tiles + 2 PV
  accumulators + 2 filler tiles.

  - Scores for the two heads of a pair go into one [128,1024] PSUM tile
    (two banks); their K=64 matmuls land in disjoint PE row groups
    (tile_position auto-derives from lhsT base partition) so they can
    overlap on the array. One [128,1024] exp per (pr, kt) on ACT.
  - Causal mask: multiplicative 0/1 bf16 mask on the exp tile (diagonal
    128-tiles only), broadcast across the two heads in one DVE op.
  - PV matmul: V gets a ones column (M=65) so row 64 of the PV psum
    accumulates the softmax denominator for free.
  - Epilogue per (chunk, pair): DMA the two denominator rows out of PSUM,
    one reciprocal_approx_fast [2,512], DMA-broadcast each row to 64
    partitions, two DVE multiplies into yT (the upper-head half staged
    through a base-0 temp + DMA because elementwise ops cannot change
    partition base).
  - QKV bias rides DVE tensor_scalar_add (PSUM->SBUF cast+bias in one op)
    so ACT does nothing but exp.
"""

import os
import sys

if "/opt/trn_rl_repo" not in sys.path:
    sys.path.insert(0, "/opt/trn_rl_repo")

import numpy as np
import ml_dtypes

import concourse.bass as bass
import concourse.mybir as mybir
import concourse.tile as tile

F32 = mybir.dt.float32
BF16 = mybir.dt.bfloat16

B, L, D = 4, 2048, 1024
H, DH = 16, 64
NCORES = 8
G = 2                 # head groups (cores per batch)
HPC = H // G          # heads per core = 8
DQ = HPC * DH         # per-core q/k/v width = 512
PAIRS = HPC // 2      # 128-partition head pairs = 4
CHUNK = 512           # query-chunk (matmul free dim)
NCH = L // CHUNK      # 4
KT = D // 128         # 8 k-tiles over d_model
LT = L // 128         # 16 l-tiles
VW = DH + 1           # V columns per head incl. ones column = 65

LAST_RESULTS = None   # test harness reads perf fields from here


def legalize_bir_waits(bir_json: bytes) -> bytes:
    """Split multi-wait sync_infos into standalone EventSemaphore instrs.

    This container's walrus codegen accepts at most ONE sync wait per
    instruction (two for EventSemaphore), but Tile's sem assigner happily
    attaches several.  For every instruction carrying N>1 waits, keep one
    and hoist the rest onto EventSemaphore instructions inserted directly
    before it on the same engine (same block), which preserves each
    engine's program order and therefore the sync semantics.
    """
    import json as _json

    j = _json.loads(bir_json)
    uid = [0]
    for fn in j["functions"]:
        for blk in fn["blocks"]:
            out_insts = []
            for inst in blk["instructions"]:
                si = inst.get("sync_info")
                waits = (si or {}).get("on_wait") or []
                cap = 2 if inst.get("opcode") == "EventSemaphore" else 1
                if len(waits) > cap:
                    extra, keep = waits[:-cap], waits[-cap:]
                    for i in range(0, len(extra), 2):
                        uid[0] += 1
                        out_insts.append(
                            {
                                "name": f"antwaitfix-{uid[0]}",
                                "opcode": "EventSemaphore",
                                "engine": inst["engine"],
                                "ins": [],
                                "outs": [],
                                "debug": inst.get("debug", 0),
                                "sync_info": {
                                    "on_wait": extra[i : i + 2],
                                    "on_update": [],
                                },
                            }
                        )
                    si["on_wait"] = keep
                out_insts.append(inst)
            blk["instructions"] = out_insts
    return _json.dumps(j).encode()


def build_module():
    nc = bass.Bass(use_seq_codegen=True)

    xT = nc.declare_dram_parameter("xT", [D, L], BF16, isOutput=False)
    wq = nc.declare_dram_parameter("wq", [D, DQ], BF16, isOutput=False)
    wk = nc.declare_dram_parameter("wk", [D, DQ], BF16, isOutput=False)
    wv = nc.declare_dram_parameter("wv", [D, DQ], BF16, isOutput=False)
    wo = nc.declare_dram_parameter("wo", [DQ, D], BF16, isOutput=False)
    bq = nc.declare_dram_parameter("bq", [128, PAIRS], F32, isOutput=False)
    bk = nc.declare_dram_parameter("bk", [128, PAIRS], F32, isOutput=False)
    bv = nc.declare_dram_parameter("bv", [128, DQ], F32, isOutput=False)
    cosT = nc.declare_dram_parameter("cosT", [128, L], BF16, isOutput=False)
    sinT = nc.declare_dram_parameter("sinT", [128, L], BF16, isOutput=False)
    maskb = nc.declare_dram_parameter("maskb", [128, 896], BF16, isOutput=False)
    out = nc.declare_dram_parameter("out", [L, D], F32, isOutput=True)

    with tile.TileContext(nc) as tc:
        with (
            tc.tile_pool(name="const", bufs=1) as cp,
            tc.tile_pool(name="acts", bufs=1) as ap,
            tc.tile_pool(name="work", bufs=4) as wp,
            tc.tile_pool(name="sc", bufs=2, space="PSUM") as scp,
            tc.tile_pool(name="pv", bufs=2, space="PSUM") as pvp,
            tc.tile_pool(name="fp", bufs=2, space="PSUM") as fpp,
        ):
            # ---- constant / activation loads (split for DMA-queue spread)
            xT_sb = ap.tile([128, KT, L], BF16)
            for kt in range(KT):
                nc.sync.dma_start(
                    xT_sb[:, kt, :],
                    xT.rearrange("(kt p) l -> p kt l", p=128)[:, kt, :],
                )
            wq_sb = cp.tile([128, KT, DQ], BF16)
            wk_sb = cp.tile([128, KT, DQ], BF16)
            wv_sb = cp.tile([128, KT, DQ], BF16)
            for kt in range(KT):
                nc.sync.dma_start(
                    wq_sb[:, kt, :], wq.rearrange("(kt p) m -> p kt m", p=128)[:, kt, :]
                )
                nc.sync.dma_start(
                    wk_sb[:, kt, :], wk.rearrange("(kt p) m -> p kt m", p=128)[:, kt, :]
                )
                nc.sync.dma_start(
                    wv_sb[:, kt, :], wv.rearrange("(kt p) m -> p kt m", p=128)[:, kt, :]
                )
            wo_sb = cp.tile([128, PAIRS, D], BF16)
            for pr in range(PAIRS):
                nc.sync.dma_start(
                    wo_sb[:, pr, :], wo.rearrange("(pr p) c -> p pr c", p=128)[:, pr, :]
                )
            bq_sb = cp.tile([128, PAIRS], F32)
            bk_sb = cp.tile([128, PAIRS], F32)
            bv_sb = cp.tile([128, DQ], F32)
            cos_sb = cp.tile([128, L], BF16)
            sin_sb = cp.tile([128, L], BF16)
            mask_sb = cp.tile([128, 896], BF16)
            nc.sync.dma_start(bq_sb[:], bq[:])
            nc.sync.dma_start(bk_sb[:], bk[:])
            nc.sync.dma_start(bv_sb[:], bv[:])
            nc.sync.dma_start(cos_sb[:], cosT[:])
            nc.sync.dma_start(sin_sb[:], sinT[:])
            nc.sync.dma_start(mask_sb[:], maskb[:])

            # Selector rows for the denominator-broadcast matmuls:
            # sel[:, 0, :] = [1]*64 + [0]*64, sel[:, 1, :] = its complement.
            # memset can't encode a float32r immediate; memset f32 then
            # copy-convert (bitwise identical) into the f32r tile.
            sel_f32 = cp.tile([128, 2, 128], F32)
            nc.vector.memset(sel_f32[:, 0, 0:64], 1.0)
            nc.vector.memset(sel_f32[:, 0, 64:128], 0.0)
            nc.vector.memset(sel_f32[:, 1, 0:64], 0.0)
            nc.vector.memset(sel_f32[:, 1, 64:128], 1.0)
            sel_sb = cp.tile([128, 2, 128], mybir.dt.float32r)
            with nc.allow_low_precision(reason="f32r selectors for bcast mm"):
                nc.vector.tensor_copy(sel_sb[:], sel_f32[:])

            qT_sb = ap.tile([128, PAIRS, L], BF16)
            kT_sb = ap.tile([128, PAIRS, L], BF16)
            v_sb = ap.tile([128, LT, HPC * VW], BF16)
            yT_sb = ap.tile([128, PAIRS, L], BF16)
            # ones columns of V, set once for all l-tiles
            v4 = v_sb.rearrange("p lt (h c) -> p lt h c", c=VW)
            nc.vector.memset(v4[:, :, :, DH:VW], 1.0)

            def proj_chunk(c):
                """Project q/k/v for query/key chunk c and apply RoPE."""
                cs = slice(c * CHUNK, (c + 1) * CHUNK)
                for mt in range(PAIRS):
                    for dst, w_sb, b_sb in (
                        (qT_sb, wq_sb, bq_sb),
                        (kT_sb, wk_sb, bk_sb),
                    ):
                        ps = fpp.tile([128, CHUNK], F32, tag="fp", name=f"qk_{c}_{mt}")
                        for kt in range(KT):
                            nc.tensor.matmul(
                                ps[:],
                                w_sb[:, kt, mt * 128 : (mt + 1) * 128],
                                xT_sb[:, kt, cs],
                                start=(kt == 0),
                                stop=(kt == KT - 1),
                            )
                        nc.vector.tensor_scalar_add(
                            dst[:, mt, cs], ps[:], b_sb[:, mt : mt + 1]
                        )
                for lt in range(4 * c, 4 * c + 4):
                    ps = fpp.tile([128, CHUNK], F32, tag="fp", name=f"v_{lt}")
                    for kt in range(KT):
                        nc.tensor.matmul(
                            ps[:],
                            xT_sb[:, kt, lt * 128 : (lt + 1) * 128],
                            wv_sb[:, kt, :],
                            start=(kt == 0),
                            stop=(kt == KT - 1),
                        )
                    vdst = v_sb[:, lt, :].rearrange("p (h c) -> p h c", c=VW)
                    nc.vector.tensor_add(vdst[:, :, 0:DH], ps[:], bv_sb[:])
                # RoPE in place on the chunk's qT/kT columns
                for dst in (qT_sb, kT_sb):
                    for mt in range(PAIRS):
                        t = dst[:, mt, cs]
                        swp = wp.tile([128, CHUNK], BF16, tag="swp")
                        for i in range(4):
                            j = i ^ 1
                            nc.sync.dma_start(
                                swp[i * 32 : (i + 1) * 32, :],
                                t[j * 32 : (j + 1) * 32, :],
                            )
                        nc.vector.tensor_mul(swp[:], swp[:], sin_sb[:, cs])
                        nc.vector.tensor_mul(t, t, cos_sb[:, cs])
                        nc.vector.tensor_add(t, t, swp[:])

            def attn_chunk(c):
                q0 = c * CHUNK
                n_lk = 4 * (c + 1)
                for pr in range(PAIRS):
                    ys = [
                        pvp.tile([128, CHUNK], F32, tag="pv", name=f"ys_{c}_{pr}_{hh}")
                        for hh in range(2)
                    ]
                    for kt in range(n_lk):
                        k0 = kt * 128
                        sct = scp.tile(
                            [128, 2, CHUNK], F32, tag="sc", name=f"sc_{c}_{pr}_{kt}"
                        )
                        for hh in range(2):
                            nc.tensor.matmul(
                                sct[:, hh, :],
                                kT_sb[hh * 64 : (hh + 1) * 64, pr, k0 : k0 + 128],
                                qT_sb[hh * 64 : (hh + 1) * 64, pr, q0 : q0 + CHUNK],
                                start=True,
                                stop=True,
                            )
                        ex = wp.tile(
                            [128, 2, CHUNK], BF16, tag="ex", name=f"ex_{c}_{pr}_{kt}"
                        )
                        nc.scalar.activation(
                            ex[:],
                            sct[:],
                            mybir.ActivationFunctionType.Exp,
                            scale=float(1.0 / np.sqrt(DH)),
                        )
                        if k0 >= q0:
                            s = 384 - (k0 - q0)
                            mbc = (
                                mask_sb[:, s : s + CHUNK]
                                .unsqueeze(1)
                                .broadcast_to([128, 2, CHUNK])
                            )
                            nc.vector.tensor_mul(ex[:], ex[:], mbc)
                        for hh in range(2):
                            h = 2 * pr + hh
                            nc.tensor.matmul(
                                ys[hh][0:VW, :],
                                v_sb[:, kt, h * VW : (h + 1) * VW],
                                ex[:, hh, :],
                                start=(kt == 0),
                                stop=(kt == n_lk - 1),
                            )
                    # epilogue: normalize by the denominator row (row 64).
                    # PSUM can't feed DMA or matmul-rhs, so: GpSimd copies the
                    # raw denominator rows PSUM->SBUF with f32r rounding (the
                    # verifier demands an f32r-rounding producer), two f32r
                    # ones-matmuls broadcast them to 64 partitions each
                    # (hh0 -> psum rows 0:64, hh1 -> 64:128), and ONE
                    # reciprocal_approx_fast [128,512] PSUM->SBUF computes the
                    # reciprocal while staging to SBUF. A SBUF->SBUF DMA moves
                    # the hh1 half down to base 0 (elementwise engines cannot
                    # change partition base).
                    den_r = wp.tile(
                        [128, 2, CHUNK], mybir.dt.float32r, tag="denr", bufs=2,
                        name=f"denr_{c}_{pr}",
                    )
                    with nc.allow_low_precision(reason="f32r denom rounding"):
                        for hh in range(2):
                            nc.vector.tensor_copy(
                                den_r[64:65, hh, :], ys[hh][64:65, :]
                            )
                    bc_ps = fpp.tile([128, CHUNK], F32, tag="fp", name=f"bc_{c}_{pr}")
                    for hh in range(2):
                        nc.tensor.matmul(
                            bc_ps[:],
                            sel_sb[64:65, hh, :],
                            den_r[64:65, hh, :],
                            start=(hh == 0),
                            stop=(hh == 1),
                        )
                    # 1/x as exp(-ln(x)) on ACT (both funcs share one table
                    # set); vector.reciprocal costs 6.5ns/elem on DVE and the
                    # custom-DVE approx ops don't survive this walrus.
                    lnb = wp.tile([128, CHUNK], F32, tag="lnb", bufs=2,
                                  name=f"lnb_{c}_{pr}")
                    nc.scalar.activation(
                        lnb[:], bc_ps[:], mybir.ActivationFunctionType.Ln
                    )
                    bcs = wp.tile([128, CHUNK], F32, tag="bcs", bufs=2,
                                  name=f"bcs_{c}_{pr}")
                    nc.scalar.activation(
                        bcs[:], lnb[:], mybir.ActivationFunctionType.Exp,
                        scale=-1.0,
                    )
                    bcs1 = wp.tile([64, CHUNK], F32, tag="bcs1", bufs=2,
                                   name=f"bcs1_{c}_{pr}")
                    nc.sync.dma_start(bcs1[:], bcs[64:128, :])
                    nc.vector.tensor_mul(
                        yT_sb[0:64, pr, q0 : q0 + CHUNK], ys[0][0:64, :], bcs[0:64, :]
                    )
                    yt = wp.tile([64, CHUNK], BF16, tag="yt", name=f"yt_{c}_{pr}")
                    nc.vector.tensor_mul(yt[:], ys[1][0:64, :], bcs1[:])
                    nc.sync.dma_start(yT_sb[64:128, pr, q0 : q0 + CHUNK], yt[:])

            def outproj_chunk(c):
                for lt in range(4 * c, 4 * c + 4):
                    for cc in range(2):
                        ps = fpp.tile([128, CHUNK], F32, tag="fp", name=f"op_{lt}_{cc}")
                        for pr in range(PAIRS):
                            nc.tensor.matmul(
                                ps[:],
                                yT_sb[:, pr, lt * 128 : (lt + 1) * 128],
                                wo_sb[:, pr, cc * CHUNK : (cc + 1) * CHUNK],
                                start=(pr == 0),
                                stop=(pr == PAIRS - 1),
                            )
                        ob = wp.tile([128, CHUNK], F32, tag="ob", name=f"ob_{lt}_{cc}")
                        nc.vector.tensor_copy(ob[:], ps[:])
                        nc.sync.dma_start(
                            out[
                                lt * 128 : (lt + 1) * 128,
                                cc * CHUNK : (cc + 1) * CHUNK,
                            ],
                            ob[:],
                        )

            proj_chunk(0)
            for c in range(NCH):
                attn_chunk(c)
                if c < NCH - 1:
                    proj_chunk(c + 1)
                outproj_chunk(c)
    return nc


def _rope_tables():
    inv_freq = (1.0 / (10000.0 ** (np.arange(0, DH, 2, dtype=np.float32) / DH))).astype(
        np.float32
    )
    t = np.arange(L, dtype=np.float32)
    freqs = np.einsum("l,d->ld", t, inv_freq).astype(np.float32)  # (L, 32)
    emb = np.concatenate([freqs, freqs], axis=-1)                 # (L, 64)
    cos = np.cos(emb).astype(np.float32)
    sin = np.sin(emb).astype(np.float32)
    cosT = cos.T                                   # (64, L)
    sinT = sin.T.copy()
    sinT[0:32] = -sinT[0:32]                       # fold rotate_half sign
    cos128 = np.tile(cosT, (2, 1))                 # (128, L)
    sin128 = np.tile(sinT, (2, 1))
    return cos128, sin128


def _mask_big():
    # maskb[p, j] = 1.0 iff p <= j - 384 (slice at s = 384-delta gives the
    # diagonal-tile mask "p <= f - delta")
    p = np.arange(128)[:, None]
    j = np.arange(896)[None, :]
    return (p <= j - 384).astype(np.float32)


def _bf16(a):
    return np.asarray(a, dtype=np.float32).astype(ml_dtypes.bfloat16)


_COMPILED = None


def _ensure_trace_hook() -> bool:
    """Install the axon NTFF profile hook if the boot shim couldn't.

    The image's `antenv` stub lacks `axon_hooks`, so bass_utils' trace
    path crashes on import. Synthesize the module and wire in the ctypes
    hook from trn_agent_boot. Returns True iff tracing is usable.
    """
    try:
        from antenv.axon_hooks import get_axon_ntff_profile_hook  # noqa: F401

        return True
    except ImportError:
        pass
    try:
        import types

        import antenv
        import trn_agent_boot.trn_boot as tb

        mod = types.ModuleType("antenv.axon_hooks")
        _hook = [None]
        mod.set_axon_ntff_profile_hook = lambda h: _hook.__setitem__(0, h)
        mod.get_axon_ntff_profile_hook = lambda: _hook[0]
        sys.modules["antenv.axon_hooks"] = mod
        antenv.axon_hooks = mod
        mod.set_axon_ntff_profile_hook(
            tb._ntff_profile_via_ctypes("/opt/axon/libaxon_pjrt.so")
        )
        return True
    except Exception:
        return False


def kernel(x, pad_mask, W_qkv, b_qkv, W_out, b_out):
    global LAST_RESULTS, _COMPILED
    from concourse.bass_utils import run_bass_kernel_spmd

    x = np.asarray(x, dtype=np.float32)
    W_qkv = np.asarray(W_qkv, dtype=np.float32)
    b_qkv = np.asarray(b_qkv, dtype=np.float32)
    W_out = np.asarray(W_out, dtype=np.float32)
    b_out = np.asarray(b_out, dtype=np.float32)

    cos128, sin128 = _rope_tables()
    maskb = _mask_big()

    in_maps = []
    for core in range(NCORES):
        b, g = core // G, core % G
        sl = slice(g * DQ, (g + 1) * DQ)
        wq = W_qkv[:, 0 * D : 1 * D][:, sl]
        wk = W_qkv[:, 1 * D : 2 * D][:, sl]
        wv = W_qkv[:, 2 * D : 3 * D][:, sl]
        bqv = b_qkv[0 * D : 1 * D][sl]
        bkv = b_qkv[1 * D : 2 * D][sl]
        bvv = b_qkv[2 * D : 3 * D][sl]
        in_maps.append(
            {
                "xT": _bf16(x[b].T),
                "wq": _bf16(wq),
                "wk": _bf16(wk),
                "wv": _bf16(wv),
                "wo": _bf16(W_out[sl, :]),
                "bq": np.ascontiguousarray(bqv.reshape(PAIRS, 128).T),
                "bk": np.ascontiguousarray(bkv.reshape(PAIRS, 128).T),
                "bv": np.tile(bvv[None, :], (128, 1)).astype(np.float32),
                "cosT": _bf16(cos128),
                "sinT": _bf16(sin128),
                "maskb": _bf16(maskb),
            }
        )

    if _COMPILED is None:
        nc = build_module()
        fixed = legalize_bir_waits(nc.to_json_bytes())
        nc.to_json_bytes = lambda: fixed  # bass2jax ships this BIR to walrus
        _COMPILED = nc
    nc = _COMPILED

    res = run_bass_kernel_spmd(
        nc,
        in_maps,
        core_ids=list(range(NCORES)),
        trace=bool(os.environ.get("BASS_TRACE")) and _ensure_trace_hook(),
    )
    LAST_RESULTS = res

    out = np.zeros((B, L, D), dtype=np.float32)
    for core in range(NCORES):
        out[core // G] += np.asarray(res.results[core]["out"], dtype=np.float32)
    out += b_out[None, None, :]
    return out


# revision 15
# speedup vs baseline: 1.3194x; 1.0241x over previous
"""Causal self-attention with RoPE on 8 Trainium2 NeuronCores.

Sharding: batch x head-group. Core c handles batch b = c//2 and head group
g = c%2 (8 of 16 heads). Each core runs the full per-(batch, head-group)
pipeline on device; the host sums the two partial output projections per
batch and adds b_out.

v2 layout (chunk-pipelined for PE warmth):
  The TRN2 PE clock-gates to 1.2 GHz after any ~3.4us idle window and only
  reaches 2.4 GHz under sustained work, so the whole kernel is emitted as a
  single software-pipelined stream: QKV projection for chunk c+1 and the
  output projection for chunk c are "filler" PE work that the Tile list
  scheduler pulls into the gaps of chunk c's attention (which is paced by
  ACT exp). PSUM budget (8 banks): 2 x [128,1024] score tiles + 2 PV
  accumulators + 2 filler tiles.

  - Scores for the two heads of a pair go into one [128,1024] PSUM tile
    (two banks); their K=64 matmuls land in disjoint PE row groups
    (tile_position auto-derives from lhsT base partition) so they can
    overlap on the array. One [128,1024] exp per (pr, kt) on ACT.
  - Causal mask: multiplicative 0/1 bf16 mask on the exp tile (diagonal
    128-tiles only), broadcast across the two heads in one DVE op.
  - PV matmul: V gets a ones column (M=65) so row 64 of the PV psum
    accumulates the softmax denominator for free.
  - Epilogue per (chunk, pair): DMA the two denominator rows out of PSUM,
    one reciprocal_approx_fast [2,512], DMA-broadcast each row to 64
    partitions, two DVE multiplies into yT (the upper-head half staged
    through a base-0 temp + DMA because elementwise ops cannot change
    partition base).
  - QKV bias rides DVE tensor_scalar_add (PSUM->SBUF cast+bias in one op)
    so ACT does nothing but exp.
"""

import os
import sys

if "/opt/trn_rl_repo" not in sys.path:
    sys.path.insert(0, "/opt/trn_rl_repo")

import numpy as np
import ml_dtypes

import concourse.bass as bass
import concourse.mybir as mybir
import concourse.tile as tile

F32 = mybir.dt.float32
BF16 = mybir.dt.bfloat16

B, L, D = 4, 2048, 1024
H, DH = 16, 64
NCORES = 8
G = 2                 # head groups (cores per batch)
HPC = H // G          # heads per core = 8
DQ = HPC * DH         # per-core q/k/v width = 512
PAIRS = HPC // 2      # 128-partition head pairs = 4
CHUNK = 512           # query-chunk (matmul free dim)
NCH = L // CHUNK      # 4
KT = D // 128         # 8 k-tiles over d_model
LT = L // 128         # 16 l-tiles
VW = DH + 1           # V columns per head incl. ones column = 65

LAST_RESULTS = None   # test harness reads perf fields from here


def legalize_bir_waits(bir_json: bytes) -> bytes:
    """Split multi-wait sync_infos into standalone EventSemaphore instrs.

    This container's walrus codegen accepts at most ONE sync wait per
    instruction (two for EventSemaphore), but Tile's sem assigner happily
    attaches several.  For every instruction carrying N>1 waits, keep one
    and hoist the rest onto EventSemaphore instructions inserted directly
    before it on the same engine (same block), which preserves each
    engine's program order and therefore the sync semantics.
    """
    import json as _json

    j = _json.loads(bir_json)
    uid = [0]
    for fn in j["functions"]:
        for blk in fn["blocks"]:
            out_insts = []
            for inst in blk["instructions"]:
                si = inst.get("sync_info")
                waits = (si or {}).get("on_wait") or []
                cap = 2 if inst.get("opcode") == "EventSemaphore" else 1
                if len(waits) > cap:
                    extra, keep = waits[:-cap], waits[-cap:]
                    for i in range(0, len(extra), 2):
                        uid[0] += 1
                        out_insts.append(
                            {
                                "name": f"antwaitfix-{uid[0]}",
                                "opcode": "EventSemaphore",
                                "engine": inst["engine"],
                                "ins": [],
                                "outs": [],
                                "debug": inst.get("debug", 0),
                                "sync_info": {
                                    "on_wait": extra[i : i + 2],
                                    "on_update": [],
                                },
                            }
                        )
                    si["on_wait"] = keep
                out_insts.append(inst)
            blk["instructions"] = out_insts
    return _json.dumps(j).encode()


def build_module():
    nc = bass.Bass(use_seq_codegen=True)

    xT = nc.declare_dram_parameter("xT", [D, L], BF16, isOutput=False)
    wq = nc.declare_dram_parameter("wq", [D, DQ], BF16, isOutput=False)
    wk = nc.declare_dram_parameter("wk", [D, DQ], BF16, isOutput=False)
    wv = nc.declare_dram_parameter("wv", [D, DQ], BF16, isOutput=False)
    wo = nc.declare_dram_parameter("wo", [DQ, D], BF16, isOutput=False)
    bq = nc.declare_dram_parameter("bq", [128, PAIRS], F32, isOutput=False)
    bk = nc.declare_dram_parameter("bk", [128, PAIRS], F32, isOutput=False)
    bv = nc.declare_dram_parameter("bv", [128, DQ], F32, isOutput=False)
    cosT = nc.declare_dram_parameter("cosT", [128, L], BF16, isOutput=False)
    sinT = nc.declare_dram_parameter("sinT", [128, L], BF16, isOutput=False)
    maskb = nc.declare_dram_parameter("maskb", [128, 896], BF16, isOutput=False)
    out = nc.declare_dram_parameter("out", [L, D], F32, isOutput=True)

    with tile.TileContext(nc) as tc:
        with (
            tc.tile_pool(name="const", bufs=1) as cp,
            tc.tile_pool(name="acts", bufs=1) as ap,
            tc.tile_pool(name="work", bufs=4) as wp,
            tc.tile_pool(name="sc", bufs=2, space="PSUM") as scp,
            tc.tile_pool(name="pv", bufs=2, space="PSUM") as pvp,
            tc.tile_pool(name="fp", bufs=2, space="PSUM") as fpp,
        ):
            # ---- input loads, ordered so chunk-0 compute starts ~1us in:
            # q/k weights + the first xT column-chunk first, then the rest
            # streams in under compute. xT is loaded column-chunked so the
            # first projection isn't gated on the full 4MB activation load.
            xT_sb = ap.tile([128, KT, L], BF16)
            wq_sb = cp.tile([128, KT, DQ], BF16)
            wk_sb = cp.tile([128, KT, DQ], BF16)
            wv_sb = cp.tile([128, KT, DQ], BF16)
            xTr = xT.rearrange("(kt p) l -> p kt l", p=128)
            for kt in range(KT):
                nc.sync.dma_start(
                    wq_sb[:, kt, :], wq.rearrange("(kt p) m -> p kt m", p=128)[:, kt, :]
                )
                nc.sync.dma_start(
                    xT_sb[:, kt, 0:CHUNK], xTr[:, kt, 0:CHUNK]
                )
            bq_sb = cp.tile([128, PAIRS], F32)
            bk_sb = cp.tile([128, PAIRS], F32)
            bv_sb = cp.tile([128, DQ], F32)
            cos_sb = cp.tile([128, L], BF16)
            sin_sb = cp.tile([128, L], BF16)
            mask_sb = cp.tile([128, 896], BF16)
            nc.sync.dma_start(bq_sb[:], bq[:])
            nc.sync.dma_start(bk_sb[:], bk[:])
            nc.sync.dma_start(cos_sb[:], cosT[:])
            nc.sync.dma_start(sin_sb[:], sinT[:])
            for kt in range(KT):
                nc.sync.dma_start(
                    wk_sb[:, kt, :], wk.rearrange("(kt p) m -> p kt m", p=128)[:, kt, :]
                )
                nc.sync.dma_start(
                    wv_sb[:, kt, :], wv.rearrange("(kt p) m -> p kt m", p=128)[:, kt, :]
                )
            nc.sync.dma_start(bv_sb[:], bv[:])
            nc.sync.dma_start(mask_sb[:], maskb[:])
            for c in range(1, NCH):
                for kt in range(KT):
                    nc.sync.dma_start(
                        xT_sb[:, kt, c * CHUNK : (c + 1) * CHUNK],
                        xTr[:, kt, c * CHUNK : (c + 1) * CHUNK],
                    )
            wo_sb = cp.tile([128, PAIRS, D], BF16)
            for pr in range(PAIRS):
                nc.sync.dma_start(
                    wo_sb[:, pr, :], wo.rearrange("(pr p) c -> p pr c", p=128)[:, pr, :]
                )

            # Selector rows for the denominator-broadcast matmuls:
            # sel[:, 0, :] = [1]*64 + [0]*64, sel[:, 1, :] = its complement.
            # memset can't encode a float32r immediate; memset f32 then
            # copy-convert (bitwise identical) into the f32r tile.
            sel_f32 = cp.tile([128, 2, 128], F32)
            nc.vector.memset(sel_f32[:, 0, 0:64], 1.0)
            nc.vector.memset(sel_f32[:, 0, 64:128], 0.0)
            nc.vector.memset(sel_f32[:, 1, 0:64], 0.0)
            nc.vector.memset(sel_f32[:, 1, 64:128], 1.0)
            sel_sb = cp.tile([128, 2, 128], mybir.dt.float32r)
            with nc.allow_low_precision(reason="f32r selectors for bcast mm"):
                nc.vector.tensor_copy(sel_sb[:], sel_f32[:])

            qT_sb = ap.tile([128, PAIRS, L], BF16)
            kT_sb = ap.tile([128, PAIRS, L], BF16)
            v_sb = ap.tile([128, LT, HPC * VW], BF16)
            yT_sb = ap.tile([128, PAIRS, L], BF16)
            # ones columns of V, set once for all l-tiles
            v4 = v_sb.rearrange("p lt (h c) -> p lt h c", c=VW)
            nc.vector.memset(v4[:, :, :, DH:VW], 1.0)

            def proj_chunk(c):
                """Project q/k/v for query/key chunk c and apply RoPE."""
                cs = slice(c * CHUNK, (c + 1) * CHUNK)
                for mt in range(PAIRS):
                    for dst, w_sb, b_sb in (
                        (qT_sb, wq_sb, bq_sb),
                        (kT_sb, wk_sb, bk_sb),
                    ):
                        ps = fpp.tile([128, CHUNK], F32, tag="fp", name=f"qk_{c}_{mt}")
                        for kt in range(KT):
                            nc.tensor.matmul(
                                ps[:],
                                w_sb[:, kt, mt * 128 : (mt + 1) * 128],
                                xT_sb[:, kt, cs],
                                start=(kt == 0),
                                stop=(kt == KT - 1),
                            )
                        nc.vector.tensor_scalar_add(
                            dst[:, mt, cs], ps[:], b_sb[:, mt : mt + 1]
                        )
                for lt in range(4 * c, 4 * c + 4):
                    ps = fpp.tile([128, CHUNK], F32, tag="fp", name=f"v_{lt}")
                    for kt in range(KT):
                        nc.tensor.matmul(
                            ps[:],
                            xT_sb[:, kt, lt * 128 : (lt + 1) * 128],
                            wv_sb[:, kt, :],
                            start=(kt == 0),
                            stop=(kt == KT - 1),
                        )
                    vdst = v_sb[:, lt, :].rearrange("p (h c) -> p h c", c=VW)
                    nc.vector.tensor_add(vdst[:, :, 0:DH], ps[:], bv_sb[:])
                # RoPE in place on the chunk's qT/kT columns
                for dst in (qT_sb, kT_sb):
                    for mt in range(PAIRS):
                        t = dst[:, mt, cs]
                        swp = wp.tile([128, CHUNK], BF16, tag="swp")
                        for i in range(4):
                            j = i ^ 1
                            nc.sync.dma_start(
                                swp[i * 32 : (i + 1) * 32, :],
                                t[j * 32 : (j + 1) * 32, :],
                            )
                        nc.vector.tensor_mul(swp[:], swp[:], sin_sb[:, cs])
                        nc.vector.tensor_mul(t, t, cos_sb[:, cs])
                        nc.vector.tensor_add(t, t, swp[:])

            def attn_chunk(c):
                q0 = c * CHUNK
                n_lk = 4 * (c + 1)
                for pr in range(PAIRS):
                    ys = [
                        pvp.tile([128, CHUNK], F32, tag="pv", name=f"ys_{c}_{pr}_{hh}")
                        for hh in range(2)
                    ]
                    for kt in range(n_lk):
                        k0 = kt * 128
                        sct = scp.tile(
                            [128, 2, CHUNK], F32, tag="sc", name=f"sc_{c}_{pr}_{kt}"
                        )
                        for hh in range(2):
                            nc.tensor.matmul(
                                sct[:, hh, :],
                                kT_sb[hh * 64 : (hh + 1) * 64, pr, k0 : k0 + 128],
                                qT_sb[hh * 64 : (hh + 1) * 64, pr, q0 : q0 + CHUNK],
                                start=True,
                                stop=True,
                            )
                        ex = wp.tile(
                            [128, 2, CHUNK], BF16, tag="ex", name=f"ex_{c}_{pr}_{kt}"
                        )
                        nc.scalar.activation(
                            ex[:],
                            sct[:],
                            mybir.ActivationFunctionType.Exp,
                            scale=float(1.0 / np.sqrt(DH)),
                        )
                        if k0 >= q0:
                            s = 384 - (k0 - q0)
                            mbc = (
                                mask_sb[:, s : s + CHUNK]
                                .unsqueeze(1)
                                .broadcast_to([128, 2, CHUNK])
                            )
                            nc.vector.tensor_mul(ex[:], ex[:], mbc)
                        for hh in range(2):
                            h = 2 * pr + hh
                            nc.tensor.matmul(
                                ys[hh][0:VW, :],
                                v_sb[:, kt, h * VW : (h + 1) * VW],
                                ex[:, hh, :],
                                start=(kt == 0),
                                stop=(kt == n_lk - 1),
                            )
                    # epilogue: normalize by the denominator row (row 64).
                    # PSUM can't feed DMA or matmul-rhs, so: GpSimd copies the
                    # raw denominator rows PSUM->SBUF with f32r rounding (the
                    # verifier demands an f32r-rounding producer), two f32r
                    # ones-matmuls broadcast them to 64 partitions each
                    # (hh0 -> psum rows 0:64, hh1 -> 64:128), and ONE
                    # reciprocal_approx_fast [128,512] PSUM->SBUF computes the
                    # reciprocal while staging to SBUF. A SBUF->SBUF DMA moves
                    # the hh1 half down to base 0 (elementwise engines cannot
                    # change partition base).
                    den_r = wp.tile(
                        [128, 2, CHUNK], mybir.dt.float32r, tag="denr", bufs=2,
                        name=f"denr_{c}_{pr}",
                    )
                    with nc.allow_low_precision(reason="f32r denom rounding"):
                        for hh in range(2):
                            nc.vector.tensor_copy(
                                den_r[64:65, hh, :], ys[hh][64:65, :]
                            )
                    bc_ps = fpp.tile([128, CHUNK], F32, tag="fp", name=f"bc_{c}_{pr}")
                    for hh in range(2):
                        nc.tensor.matmul(
                            bc_ps[:],
                            sel_sb[64:65, hh, :],
                            den_r[64:65, hh, :],
                            start=(hh == 0),
                            stop=(hh == 1),
                        )
                    # 1/x as exp(-ln(x)) on ACT (both funcs share one table
                    # set); vector.reciprocal costs 6.5ns/elem on DVE and the
                    # custom-DVE approx ops don't survive this walrus.
                    lnb = wp.tile([128, CHUNK], F32, tag="lnb", bufs=2,
                                  name=f"lnb_{c}_{pr}")
                    nc.scalar.activation(
                        lnb[:], bc_ps[:], mybir.ActivationFunctionType.Ln
                    )
                    bcs = wp.tile([128, CHUNK], F32, tag="bcs", bufs=2,
                                  name=f"bcs_{c}_{pr}")
                    nc.scalar.activation(
                        bcs[:], lnb[:], mybir.ActivationFunctionType.Exp,
                        scale=-1.0,
                    )
                    bcs1 = wp.tile([64, CHUNK], F32, tag="bcs1", bufs=2,
                                   name=f"bcs1_{c}_{pr}")
                    nc.sync.dma_start(bcs1[:], bcs[64:128, :])
                    nc.vector.tensor_mul(
                        yT_sb[0:64, pr, q0 : q0 + CHUNK], ys[0][0:64, :], bcs[0:64, :]
                    )
                    yt = wp.tile([64, CHUNK], BF16, tag="yt", name=f"yt_{c}_{pr}")
                    nc.vector.tensor_mul(yt[:], ys[1][0:64, :], bcs1[:])
                    nc.sync.dma_start(yT_sb[64:128, pr, q0 : q0 + CHUNK], yt[:])

            def outproj_chunk(c):
                for lt in range(4 * c, 4 * c + 4):
                    for cc in range(2):
                        ps = fpp.tile([128, CHUNK], F32, tag="fp", name=f"op_{lt}_{cc}")
                        for pr in range(PAIRS):
                            nc.tensor.matmul(
                                ps[:],
                                yT_sb[:, pr, lt * 128 : (lt + 1) * 128],
                                wo_sb[:, pr, cc * CHUNK : (cc + 1) * CHUNK],
                                start=(pr == 0),
                                stop=(pr == PAIRS - 1),
                            )
                        ob = wp.tile([128, CHUNK], F32, tag="ob", name=f"ob_{lt}_{cc}")
                        nc.vector.tensor_copy(ob[:], ps[:])
                        nc.sync.dma_start(
                            out[
                                lt * 128 : (lt + 1) * 128,
                                cc * CHUNK : (cc + 1) * CHUNK,
                            ],
                            ob[:],
                        )

            # proj(c+1) is emitted BEFORE attn(c): its PE matmuls serve as
            # filler work during attention (which is ACT-exp paced) and its
            # DVE bias/RoPE ops get priority so the next chunk's q/k/v are
            # ready the moment attention(c) drains.
            proj_chunk(0)
            proj_chunk(1)
            attn_chunk(0)
            proj_chunk(2)
            outproj_chunk(0)
            attn_chunk(1)
            proj_chunk(3)
            outproj_chunk(1)
            attn_chunk(2)
            outproj_chunk(2)
            attn_chunk(3)
            outproj_chunk(3)
    return nc


def _rope_tables():
    inv_freq = (1.0 / (10000.0 ** (np.arange(0, DH, 2, dtype=np.float32) / DH))).astype(
        np.float32
    )
    t = np.arange(L, dtype=np.float32)
    freqs = np.einsum("l,d->ld", t, inv_freq).astype(np.float32)  # (L, 32)
    emb = np.concatenate([freqs, freqs], axis=-1)                 # (L, 64)
    cos = np.cos(emb).astype(np.float32)
    sin = np.sin(emb).astype(np.float32)
    cosT = cos.T                                   # (64, L)
    sinT = sin.T.copy()
    sinT[0:32] = -sinT[0:32]                       # fold rotate_half sign
    cos128 = np.tile(cosT, (2, 1))                 # (128, L)
    sin128 = np.tile(sinT, (2, 1))
    return cos128, sin128


def _mask_big():
    # maskb[p, j] = 1.0 iff p <= j - 384 (slice at s = 384-delta gives the
    # diagonal-tile mask "p <= f - delta")
    p = np.arange(128)[:, None]
    j = np.arange(896)[None, :]
    return (p <= j - 384).astype(np.float32)


def _bf16(a):
    return np.asarray(a, dtype=np.float32).astype(ml_dtypes.bfloat16)


_COMPILED = None


def _ensure_trace_hook() -> bool:
    """Install the axon NTFF profile hook if the boot shim couldn't.

    The image's `antenv` stub lacks `axon_hooks`, so bass_utils' trace
    path crashes on import. Synthesize the module and wire in the ctypes
    hook from trn_agent_boot. Returns True iff tracing is usable.
    """
    try:
        from antenv.axon_hooks import get_axon_ntff_profile_hook  # noqa: F401

        return True
    except ImportError:
        pass
    try:
        import types

        import antenv
        import trn_agent_boot.trn_boot as tb

        mod = types.ModuleType("antenv.axon_hooks")
        _hook = [None]
        mod.set_axon_ntff_profile_hook = lambda h: _hook.__setitem__(0, h)
        mod.get_axon_ntff_profile_hook = lambda: _hook[0]
        sys.modules["antenv.axon_hooks"] = mod
        antenv.axon_hooks = mod
        mod.set_axon_ntff_profile_hook(
            tb._ntff_profile_via_ctypes("/opt/axon/libaxon_pjrt.so")
        )
        return True
    except Exception:
        return False


def kernel(x, pad_mask, W_qkv, b_qkv, W_out, b_out):
    global LAST_RESULTS, _COMPILED
    from concourse.bass_utils import run_bass_kernel_spmd

    x = np.asarray(x, dtype=np.float32)
    W_qkv = np.asarray(W_qkv, dtype=np.float32)
    b_qkv = np.asarray(b_qkv, dtype=np.float32)
    W_out = np.asarray(W_out, dtype=np.float32)
    b_out = np.asarray(b_out, dtype=np.float32)

    cos128, sin128 = _rope_tables()
    maskb = _mask_big()

    in_maps = []
    for core in range(NCORES):
        b, g = core // G, core % G
        sl = slice(g * DQ, (g + 1) * DQ)
        wq = W_qkv[:, 0 * D : 1 * D][:, sl]
        wk = W_qkv[:, 1 * D : 2 * D][:, sl]
        wv = W_qkv[:, 2 * D : 3 * D][:, sl]
        bqv = b_qkv[0 * D : 1 * D][sl]
        bkv = b_qkv[1 * D : 2 * D][sl]
        bvv = b_qkv[2 * D : 3 * D][sl]
        in_maps.append(
            {
                "xT": _bf16(x[b].T),
                "wq": _bf16(wq),
                "wk": _bf16(wk),
                "wv": _bf16(wv),
                "wo": _bf16(W_out[sl, :]),
                "bq": np.ascontiguousarray(bqv.reshape(PAIRS, 128).T),
                "bk": np.ascontiguousarray(bkv.reshape(PAIRS, 128).T),
                "bv": np.tile(bvv[None, :], (128, 1)).astype(np.float32),
                "cosT": _bf16(cos128),
                "sinT": _bf16(sin128),
                "maskb": _bf16(maskb),
            }
        )

    if _COMPILED is None:
        nc = build_module()
        fixed = legalize_bir_waits(nc.to_json_bytes())
        nc.to_json_bytes = lambda: fixed  # bass2jax ships this BIR to walrus
        _COMPILED = nc
    nc = _COMPILED

    res = run_bass_kernel_spmd(
        nc,
        in_maps,
        core_ids=list(range(NCORES)),
        trace=bool(os.environ.get("BASS_TRACE")) and _ensure_trace_hook(),
    )
    LAST_RESULTS = res

    out = np.zeros((B, L, D), dtype=np.float32)
    for core in range(NCORES):
        out[core // G] += np.asarray(res.results[core]["out"], dtype=np.float32)
    out += b_out[None, None, :]
    return out


# revision 16
# speedup vs baseline: 1.4853x; 1.1257x over previous
"""Causal self-attention with RoPE on 8 Trainium2 NeuronCores.

Sharding: batch x head-group. Core c handles batch b = c//2 and head group
g = c%2 (8 of 16 heads). Each core runs the full per-(batch, head-group)
pipeline on device; the host sums the two partial output projections per
batch and adds b_out.

v2 layout (chunk-pipelined for PE warmth):
  The TRN2 PE clock-gates to 1.2 GHz after any ~3.4us idle window and only
  reaches 2.4 GHz under sustained work, so the whole kernel is emitted as a
  single software-pipelined stream: QKV projection for chunk c+1 and the
  output projection for chunk c are "filler" PE work that the Tile list
  scheduler pulls into the gaps of chunk c's attention (which is paced by
  ACT exp). PSUM budget (8 banks): 2 x [128,1024] score tiles + 2 PV
  accumulators + 2 filler tiles.

  - Scores for the two heads of a pair go into one [128,1024] PSUM tile
    (two banks); their K=64 matmuls land in disjoint PE row groups
    (tile_position auto-derives from lhsT base partition) so they can
    overlap on the array. One [128,1024] exp per (pr, kt) on ACT.
  - Causal mask: multiplicative 0/1 bf16 mask on the exp tile (diagonal
    128-tiles only), broadcast across the two heads in one DVE op.
  - PV matmul: V gets a ones column (M=65) so row 64 of the PV psum
    accumulates the softmax denominator for free.
  - Epilogue per (chunk, pair): DMA the two denominator rows out of PSUM,
    one reciprocal_approx_fast [2,512], DMA-broadcast each row to 64
    partitions, two DVE multiplies into yT (the upper-head half staged
    through a base-0 temp + DMA because elementwise ops cannot change
    partition base).
  - QKV bias rides DVE tensor_scalar_add (PSUM->SBUF cast+bias in one op)
    so ACT does nothing but exp.
"""

import os
import sys

if "/opt/trn_rl_repo" not in sys.path:
    sys.path.insert(0, "/opt/trn_rl_repo")

import numpy as np
import ml_dtypes

import concourse.bass as bass
import concourse.mybir as mybir
import concourse.tile as tile

F32 = mybir.dt.float32
BF16 = mybir.dt.bfloat16

B, L, D = 4, 2048, 1024
H, DH = 16, 64
NCORES = 8
G = 2                 # head groups (cores per batch)
HPC = H // G          # heads per core = 8
DQ = HPC * DH         # per-core q/k/v width = 512
PAIRS = HPC // 2      # 128-partition head pairs = 4
CHUNK = 512           # query-chunk (matmul free dim)
NCH = L // CHUNK      # 4
KT = D // 128         # 8 k-tiles over d_model
LT = L // 128         # 16 l-tiles
VW = DH + 1           # V columns per head incl. ones column = 65

LAST_RESULTS = None   # test harness reads perf fields from here


def legalize_bir_waits(bir_json: bytes) -> bytes:
    """Split multi-wait sync_infos into standalone EventSemaphore instrs.

    This container's walrus codegen accepts at most ONE sync wait per
    instruction (two for EventSemaphore), but Tile's sem assigner happily
    attaches several.  For every instruction carrying N>1 waits, keep one
    and hoist the rest onto EventSemaphore instructions inserted directly
    before it on the same engine (same block), which preserves each
    engine's program order and therefore the sync semantics.
    """
    import json as _json

    j = _json.loads(bir_json)
    uid = [0]
    for fn in j["functions"]:
        for blk in fn["blocks"]:
            out_insts = []
            for inst in blk["instructions"]:
                si = inst.get("sync_info")
                waits = (si or {}).get("on_wait") or []
                cap = 2 if inst.get("opcode") == "EventSemaphore" else 1
                if len(waits) > cap:
                    extra, keep = waits[:-cap], waits[-cap:]
                    for i in range(0, len(extra), 2):
                        uid[0] += 1
                        out_insts.append(
                            {
                                "name": f"antwaitfix-{uid[0]}",
                                "opcode": "EventSemaphore",
                                "engine": inst["engine"],
                                "ins": [],
                                "outs": [],
                                "debug": inst.get("debug", 0),
                                "sync_info": {
                                    "on_wait": extra[i : i + 2],
                                    "on_update": [],
                                },
                            }
                        )
                    si["on_wait"] = keep
                out_insts.append(inst)
            blk["instructions"] = out_insts
    return _json.dumps(j).encode()


def build_module():
    nc = bass.Bass(use_seq_codegen=True)

    xT = nc.declare_dram_parameter("xT", [D, L], BF16, isOutput=False)
    wq = nc.declare_dram_parameter("wq", [D, DQ], BF16, isOutput=False)
    wk = nc.declare_dram_parameter("wk", [D, DQ], BF16, isOutput=False)
    wv = nc.declare_dram_parameter("wv", [D, DQ], BF16, isOutput=False)
    wo = nc.declare_dram_parameter("wo", [DQ, D], BF16, isOutput=False)
    bq = nc.declare_dram_parameter("bq", [128, PAIRS], F32, isOutput=False)
    bk = nc.declare_dram_parameter("bk", [128, PAIRS], F32, isOutput=False)
    bv = nc.declare_dram_parameter("bv", [128, DQ], F32, isOutput=False)
    cosT = nc.declare_dram_parameter("cosT", [128, L], BF16, isOutput=False)
    sinT = nc.declare_dram_parameter("sinT", [128, L], BF16, isOutput=False)
    maskb = nc.declare_dram_parameter("maskb", [128, 896], BF16, isOutput=False)
    out = nc.declare_dram_parameter("out", [L, D], F32, isOutput=True)

    with tile.TileContext(nc) as tc:
        with (
            tc.tile_pool(name="const", bufs=1) as cp,
            tc.tile_pool(name="acts", bufs=1) as ap,
            tc.tile_pool(name="work", bufs=4) as wp,
            tc.tile_pool(name="sc", bufs=2, space="PSUM") as scp,
            tc.tile_pool(name="pv", bufs=2, space="PSUM") as pvp,
            tc.tile_pool(name="fp", bufs=2, space="PSUM") as fpp,
        ):
            # ---- input loads, ordered so chunk-0 compute starts ~1us in:
            # q/k weights + the first xT column-chunk first, then the rest
            # streams in under compute. xT is loaded column-chunked so the
            # first projection isn't gated on the full 4MB activation load.
            xT_sb = ap.tile([128, KT, L], BF16)
            wq_sb = cp.tile([128, KT, DQ], BF16)
            wk_sb = cp.tile([128, KT, DQ], BF16)
            wv_sb = cp.tile([128, KT, DQ], BF16)
            xTr = xT.rearrange("(kt p) l -> p kt l", p=128)
            for kt in range(KT):
                nc.sync.dma_start(
                    wq_sb[:, kt, :], wq.rearrange("(kt p) m -> p kt m", p=128)[:, kt, :]
                )
                nc.sync.dma_start(
                    xT_sb[:, kt, 0:CHUNK], xTr[:, kt, 0:CHUNK]
                )
            bq_sb = cp.tile([128, PAIRS], F32)
            bk_sb = cp.tile([128, PAIRS], F32)
            bv_sb = cp.tile([128, DQ], F32)
            cos_sb = cp.tile([128, L], BF16)
            sin_sb = cp.tile([128, L], BF16)
            mask_sb = cp.tile([128, 896], BF16)
            nc.sync.dma_start(bq_sb[:], bq[:])
            nc.sync.dma_start(bk_sb[:], bk[:])
            nc.sync.dma_start(cos_sb[:], cosT[:])
            nc.sync.dma_start(sin_sb[:], sinT[:])
            for kt in range(KT):
                nc.sync.dma_start(
                    wk_sb[:, kt, :], wk.rearrange("(kt p) m -> p kt m", p=128)[:, kt, :]
                )
                nc.sync.dma_start(
                    wv_sb[:, kt, :], wv.rearrange("(kt p) m -> p kt m", p=128)[:, kt, :]
                )
            nc.sync.dma_start(bv_sb[:], bv[:])
            nc.sync.dma_start(mask_sb[:], maskb[:])
            for c in range(1, NCH):
                for kt in range(KT):
                    nc.sync.dma_start(
                        xT_sb[:, kt, c * CHUNK : (c + 1) * CHUNK],
                        xTr[:, kt, c * CHUNK : (c + 1) * CHUNK],
                    )
            wo_sb = cp.tile([128, PAIRS, D], BF16)
            for pr in range(PAIRS):
                nc.sync.dma_start(
                    wo_sb[:, pr, :], wo.rearrange("(pr p) c -> p pr c", p=128)[:, pr, :]
                )

            # Selector rows for the denominator-broadcast matmuls:
            # sel[:, 0, :] = [1]*64 + [0]*64, sel[:, 1, :] = its complement.
            # memset can't encode a float32r immediate; memset f32 then
            # copy-convert (bitwise identical) into the f32r tile.
            sel_f32 = cp.tile([128, 2, 128], F32)
            nc.vector.memset(sel_f32[:, 0, 0:64], 1.0)
            nc.vector.memset(sel_f32[:, 0, 64:128], 0.0)
            nc.vector.memset(sel_f32[:, 1, 0:64], 0.0)
            nc.vector.memset(sel_f32[:, 1, 64:128], 1.0)
            sel_sb = cp.tile([128, 2, 128], mybir.dt.float32r)
            with nc.allow_low_precision(reason="f32r selectors for bcast mm"):
                nc.vector.tensor_copy(sel_sb[:], sel_f32[:])

            qT_sb = ap.tile([128, PAIRS, L], BF16)
            kT_sb = ap.tile([128, PAIRS, L], BF16)
            v_sb = ap.tile([128, LT, HPC * VW], BF16)
            yT_sb = ap.tile([128, PAIRS, L], BF16)
            # ones columns of V, set once for all l-tiles
            v4 = v_sb.rearrange("p lt (h c) -> p lt h c", c=VW)
            nc.vector.memset(v4[:, :, :, DH:VW], 1.0)

            def qk_group(c, mt, which):
                cs = slice(c * CHUNK, (c + 1) * CHUNK)
                dst, w_sb, b_sb = (
                    (qT_sb, wq_sb, bq_sb) if which == "q" else (kT_sb, wk_sb, bk_sb)
                )
                ps = fpp.tile(
                    [128, CHUNK], F32, tag="fp", name=f"{which}_{c}_{mt}"
                )
                for kt in range(KT):
                    nc.tensor.matmul(
                        ps[:],
                        w_sb[:, kt, mt * 128 : (mt + 1) * 128],
                        xT_sb[:, kt, cs],
                        start=(kt == 0),
                        stop=(kt == KT - 1),
                    )
                nc.vector.tensor_scalar_add(
                    dst[:, mt, cs], ps[:], b_sb[:, mt : mt + 1]
                )

            def v_group(lt):
                ps = fpp.tile([128, CHUNK], F32, tag="fp", name=f"v_{lt}")
                for kt in range(KT):
                    nc.tensor.matmul(
                        ps[:],
                        xT_sb[:, kt, lt * 128 : (lt + 1) * 128],
                        wv_sb[:, kt, :],
                        start=(kt == 0),
                        stop=(kt == KT - 1),
                    )
                vdst = v_sb[:, lt, :].rearrange("p (h c) -> p h c", c=VW)
                nc.vector.tensor_add(vdst[:, :, 0:DH], ps[:], bv_sb[:])

            def rope_group(c, mt):
                cs = slice(c * CHUNK, (c + 1) * CHUNK)
                for dst in (qT_sb, kT_sb):
                    t = dst[:, mt, cs]
                    swp = wp.tile([128, CHUNK], BF16, tag="swp",
                                  name=f"swp_{c}_{mt}")
                    for i in range(4):
                        j = i ^ 1
                        nc.sync.dma_start(
                            swp[i * 32 : (i + 1) * 32, :],
                            t[j * 32 : (j + 1) * 32, :],
                        )
                    nc.vector.tensor_mul(swp[:], swp[:], sin_sb[:, cs])
                    nc.vector.tensor_mul(t, t, cos_sb[:, cs])
                    nc.vector.tensor_add(t, t, swp[:])

            def proj_closures(c):
                fs = []
                for mt in range(PAIRS):
                    fs.append(lambda c=c, mt=mt: qk_group(c, mt, "q"))
                    fs.append(lambda c=c, mt=mt: qk_group(c, mt, "k"))
                    fs.append(lambda c=c, mt=mt: rope_group(c, mt))
                    if mt == 0:
                        for lt in range(4 * c, 4 * c + 4):
                            fs.append(lambda lt=lt: v_group(lt))
                return fs

            def outproj_group(lt, cc):
                ps = fpp.tile([128, CHUNK], F32, tag="fp", name=f"op_{lt}_{cc}")
                for pr in range(PAIRS):
                    nc.tensor.matmul(
                        ps[:],
                        yT_sb[:, pr, lt * 128 : (lt + 1) * 128],
                        wo_sb[:, pr, cc * CHUNK : (cc + 1) * CHUNK],
                        start=(pr == 0),
                        stop=(pr == PAIRS - 1),
                    )
                ob = wp.tile([128, CHUNK], F32, tag="ob", name=f"ob_{lt}_{cc}")
                nc.vector.tensor_copy(ob[:], ps[:])
                nc.sync.dma_start(
                    out[lt * 128 : (lt + 1) * 128, cc * CHUNK : (cc + 1) * CHUNK],
                    ob[:],
                )

            def outproj_closures(c):
                return [
                    lambda lt=lt, cc=cc: outproj_group(lt, cc)
                    for lt in range(4 * c, 4 * c + 4)
                    for cc in range(2)
                ]

            def attn_iter(c, pr, kt, ys, n_lk):
                q0 = c * CHUNK
                k0 = kt * 128
                sct = scp.tile(
                    [128, 2, CHUNK], F32, tag="sc", name=f"sc_{c}_{pr}_{kt}"
                )
                for hh in range(2):
                    nc.tensor.matmul(
                        sct[:, hh, :],
                        kT_sb[hh * 64 : (hh + 1) * 64, pr, k0 : k0 + 128],
                        qT_sb[hh * 64 : (hh + 1) * 64, pr, q0 : q0 + CHUNK],
                        start=True,
                        stop=True,
                    )
                ex = wp.tile(
                    [128, 2, CHUNK], BF16, tag="ex", name=f"ex_{c}_{pr}_{kt}"
                )
                nc.scalar.activation(
                    ex[:],
                    sct[:],
                    mybir.ActivationFunctionType.Exp,
                    scale=float(1.0 / np.sqrt(DH)),
                )
                if k0 >= q0:
                    s = 384 - (k0 - q0)
                    mbc = (
                        mask_sb[:, s : s + CHUNK]
                        .unsqueeze(1)
                        .broadcast_to([128, 2, CHUNK])
                    )
                    nc.vector.tensor_mul(ex[:], ex[:], mbc)
                for hh in range(2):
                    h = 2 * pr + hh
                    nc.tensor.matmul(
                        ys[hh][0:VW, :],
                        v_sb[:, kt, h * VW : (h + 1) * VW],
                        ex[:, hh, :],
                        start=(kt == 0),
                        stop=(kt == n_lk - 1),
                    )

            def attn_epilogue(c, pr, ys):
                # normalize by the denominator row (row 64). PSUM can't feed
                # DMA or matmul-rhs, so: DVE copies the raw denominator rows
                # PSUM->SBUF with f32r rounding (the verifier demands an
                # f32r-rounding producer), two accumulating selector-matmuls
                # broadcast them (hh0 -> psum rows 0:64, hh1 -> 64:128), and
                # ACT computes 1/x as exp(-ln(x)) while staging to SBUF (both
                # funcs share one table set; DVE reciprocal costs 6.5ns/elem
                # and the custom-DVE approx ops don't survive this walrus).
                # A SBUF->SBUF DMA moves the hh1 half down to base 0
                # (elementwise engines cannot change partition base).
                q0 = c * CHUNK
                den_r = wp.tile(
                    [128, 2, CHUNK], mybir.dt.float32r, tag="denr", bufs=2,
                    name=f"denr_{c}_{pr}",
                )
                with nc.allow_low_precision(reason="f32r denom rounding"):
                    for hh in range(2):
                        nc.vector.tensor_copy(
                            den_r[64:65, hh, :], ys[hh][64:65, :]
                        )
                bc_ps = fpp.tile([128, CHUNK], F32, tag="fp", name=f"bc_{c}_{pr}")
                for hh in range(2):
                    nc.tensor.matmul(
                        bc_ps[:],
                        sel_sb[64:65, hh, :],
                        den_r[64:65, hh, :],
                        start=(hh == 0),
                        stop=(hh == 1),
                    )
                lnb = wp.tile([128, CHUNK], F32, tag="lnb", bufs=2,
                              name=f"lnb_{c}_{pr}")
                nc.scalar.activation(
                    lnb[:], bc_ps[:], mybir.ActivationFunctionType.Ln
                )
                bcs = wp.tile([128, CHUNK], F32, tag="bcs", bufs=2,
                              name=f"bcs_{c}_{pr}")
                nc.scalar.activation(
                    bcs[:], lnb[:], mybir.ActivationFunctionType.Exp,
                    scale=-1.0,
                )
                bcs1 = wp.tile([64, CHUNK], F32, tag="bcs1", bufs=2,
                               name=f"bcs1_{c}_{pr}")
                nc.sync.dma_start(bcs1[:], bcs[64:128, :])
                nc.vector.tensor_mul(
                    yT_sb[0:64, pr, q0 : q0 + CHUNK], ys[0][0:64, :], bcs[0:64, :]
                )
                yt = wp.tile([64, CHUNK], BF16, tag="yt", name=f"yt_{c}_{pr}")
                nc.vector.tensor_mul(yt[:], ys[1][0:64, :], bcs1[:])
                nc.sync.dma_start(yT_sb[64:128, pr, q0 : q0 + CHUNK], yt[:])

            def attn_chunk(c, fillers):
                """Emit chunk c's attention iterations with filler groups
                interleaved evenly (priority-spreading: the list scheduler
                prefers earlier-emitted work, so clustering fillers starves
                ACT of exp work while PE grinds through them)."""
                n_lk = 4 * (c + 1)
                n_iters = PAIRS * n_lk
                pace = len(fillers) / max(n_iters, 1)
                credit = 0.0
                for pr in range(PAIRS):
                    ys = [
                        pvp.tile(
                            [128, CHUNK], F32, tag="pv", name=f"ys_{c}_{pr}_{hh}"
                        )
                        for hh in range(2)
                    ]
                    for kt in range(n_lk):
                        attn_iter(c, pr, kt, ys, n_lk)
                        credit += pace
                        while credit >= 1.0 and fillers:
                            fillers.popleft()()
                            credit -= 1.0
                    attn_epilogue(c, pr, ys)
                while fillers:
                    fillers.popleft()()

            from collections import deque

            p0 = proj_closures(0)
            # startup: emit only what attention(0, pr=0) needs (q0/k0/rope0 +
            # the four v tiles), everything else becomes interleaved filler.
            for f in p0[:7]:
                f()
            attn_chunk(0, deque(p0[7:] + proj_closures(1)))
            attn_chunk(1, deque(proj_closures(2) + outproj_closures(0)))
            attn_chunk(2, deque(proj_closures(3) + outproj_closures(1)))
            attn_chunk(3, deque(outproj_closures(2)))
            for f in outproj_closures(3):
                f()
    return nc


def _rope_tables():
    inv_freq = (1.0 / (10000.0 ** (np.arange(0, DH, 2, dtype=np.float32) / DH))).astype(
        np.float32
    )
    t = np.arange(L, dtype=np.float32)
    freqs = np.einsum("l,d->ld", t, inv_freq).astype(np.float32)  # (L, 32)
    emb = np.concatenate([freqs, freqs], axis=-1)                 # (L, 64)
    cos = np.cos(emb).astype(np.float32)
    sin = np.sin(emb).astype(np.float32)
    cosT = cos.T                                   # (64, L)
    sinT = sin.T.copy()
    sinT[0:32] = -sinT[0:32]                       # fold rotate_half sign
    cos128 = np.tile(cosT, (2, 1))                 # (128, L)
    sin128 = np.tile(sinT, (2, 1))
    return cos128, sin128


def _mask_big():
    # maskb[p, j] = 1.0 iff p <= j - 384 (slice at s = 384-delta gives the
    # diagonal-tile mask "p <= f - delta")
    p = np.arange(128)[:, None]
    j = np.arange(896)[None, :]
    return (p <= j - 384).astype(np.float32)


def _bf16(a):
    return np.asarray(a, dtype=np.float32).astype(ml_dtypes.bfloat16)


_COMPILED = None


def _ensure_trace_hook() -> bool:
    """Install the axon NTFF profile hook if the boot shim couldn't.

    The image's `antenv` stub lacks `axon_hooks`, so bass_utils' trace
    path crashes on import. Synthesize the module and wire in the ctypes
    hook from trn_agent_boot. Returns True iff tracing is usable.
    """
    try:
        from antenv.axon_hooks import get_axon_ntff_profile_hook  # noqa: F401

        return True
    except ImportError:
        pass
    try:
        import types

        import antenv
        import trn_agent_boot.trn_boot as tb

        mod = types.ModuleType("antenv.axon_hooks")
        _hook = [None]
        mod.set_axon_ntff_profile_hook = lambda h: _hook.__setitem__(0, h)
        mod.get_axon_ntff_profile_hook = lambda: _hook[0]
        sys.modules["antenv.axon_hooks"] = mod
        antenv.axon_hooks = mod
        mod.set_axon_ntff_profile_hook(
            tb._ntff_profile_via_ctypes("/opt/axon/libaxon_pjrt.so")
        )
        return True
    except Exception:
        return False


def kernel(x, pad_mask, W_qkv, b_qkv, W_out, b_out):
    global LAST_RESULTS, _COMPILED
    from concourse.bass_utils import run_bass_kernel_spmd

    x = np.asarray(x, dtype=np.float32)
    W_qkv = np.asarray(W_qkv, dtype=np.float32)
    b_qkv = np.asarray(b_qkv, dtype=np.float32)
    W_out = np.asarray(W_out, dtype=np.float32)
    b_out = np.asarray(b_out, dtype=np.float32)

    cos128, sin128 = _rope_tables()
    maskb = _mask_big()

    in_maps = []
    for core in range(NCORES):
        b, g = core // G, core % G
        sl = slice(g * DQ, (g + 1) * DQ)
        wq = W_qkv[:, 0 * D : 1 * D][:, sl]
        wk = W_qkv[:, 1 * D : 2 * D][:, sl]
        wv = W_qkv[:, 2 * D : 3 * D][:, sl]
        bqv = b_qkv[0 * D : 1 * D][sl]
        bkv = b_qkv[1 * D : 2 * D][sl]
        bvv = b_qkv[2 * D : 3 * D][sl]
        in_maps.append(
            {
                "xT": _bf16(x[b].T),
                "wq": _bf16(wq),
                "wk": _bf16(wk),
                "wv": _bf16(wv),
                "wo": _bf16(W_out[sl, :]),
                "bq": np.ascontiguousarray(bqv.reshape(PAIRS, 128).T),
                "bk": np.ascontiguousarray(bkv.reshape(PAIRS, 128).T),
                "bv": np.tile(bvv[None, :], (128, 1)).astype(np.float32),
                "cosT": _bf16(cos128),
                "sinT": _bf16(sin128),
                "maskb": _bf16(maskb),
            }
        )

    if _COMPILED is None:
        nc = build_module()
        fixed = legalize_bir_waits(nc.to_json_bytes())
        nc.to_json_bytes = lambda: fixed  # bass2jax ships this BIR to walrus
        _COMPILED = nc
    nc = _COMPILED

    res = run_bass_kernel_spmd(
        nc,
        in_maps,
        core_ids=list(range(NCORES)),
        trace=bool(os.environ.get("BASS_TRACE")) and _ensure_trace_hook(),
    )
    LAST_RESULTS = res

    out = np.zeros((B, L, D), dtype=np.float32)
    for core in range(NCORES):
        out[core // G] += np.asarray(res.results[core]["out"], dtype=np.float32)
    out += b_out[None, None, :]
    return out


# revision 19
# speedup vs baseline: 1.5632x; 1.0525x over previous
"""Causal self-attention with RoPE on 8 Trainium2 NeuronCores.

Sharding: batch x head-group. Core c handles batch b = c//2 and head group
g = c%2 (8 of 16 heads). Each core runs the full per-(batch, head-group)
pipeline on device; the host sums the two partial output projections per
batch and adds b_out.

v2 layout (chunk-pipelined for PE warmth):
  The TRN2 PE clock-gates to 1.2 GHz after any ~3.4us idle window and only
  reaches 2.4 GHz under sustained work, so the whole kernel is emitted as a
  single software-pipelined stream: QKV projection for chunk c+1 and the
  output projection for chunk c are "filler" PE work that the Tile list
  scheduler pulls into the gaps of chunk c's attention (which is paced by
  ACT exp). PSUM budget (8 banks): 2 x [128,1024] score tiles + 2 PV
  accumulators + 2 filler tiles.

  - Scores for the two heads of a pair go into one [128,1024] PSUM tile
    (two banks); their K=64 matmuls land in disjoint PE row groups
    (tile_position auto-derives from lhsT base partition) so they can
    overlap on the array. One [128,1024] exp per (pr, kt) on ACT.
  - Causal mask: multiplicative 0/1 bf16 mask on the exp tile (diagonal
    128-tiles only), broadcast across the two heads in one DVE op.
  - PV matmul: V gets a ones column (M=65) so row 64 of the PV psum
    accumulates the softmax denominator for free.
  - Epilogue per (chunk, pair): DMA the two denominator rows out of PSUM,
    one reciprocal_approx_fast [2,512], DMA-broadcast each row to 64
    partitions, two DVE multiplies into yT (the upper-head half staged
    through a base-0 temp + DMA because elementwise ops cannot change
    partition base).
  - QKV bias rides DVE tensor_scalar_add (PSUM->SBUF cast+bias in one op)
    so ACT does nothing but exp.
"""

import os
import sys

if "/opt/trn_rl_repo" not in sys.path:
    sys.path.insert(0, "/opt/trn_rl_repo")

import numpy as np
import ml_dtypes

import concourse.bass as bass
import concourse.mybir as mybir
import concourse.tile as tile

F32 = mybir.dt.float32
BF16 = mybir.dt.bfloat16

B, L, D = 4, 2048, 1024
H, DH = 16, 64
NCORES = 8
G = 2                 # head groups (cores per batch)
HPC = H // G          # heads per core = 8
DQ = HPC * DH         # per-core q/k/v width = 512
PAIRS = HPC // 2      # 128-partition head pairs = 4
CHUNK = 512           # query-chunk (matmul free dim)
NCH = L // CHUNK      # 4
KT = D // 128         # 8 k-tiles over d_model
LT = L // 128         # 16 l-tiles
VW = DH + 1           # V columns per head incl. ones column = 65

LAST_RESULTS = None   # test harness reads perf fields from here


def legalize_bir_waits(bir_json: bytes) -> bytes:
    """Split multi-wait sync_infos into standalone EventSemaphore instrs.

    This container's walrus codegen accepts at most ONE sync wait per
    instruction (two for EventSemaphore), but Tile's sem assigner happily
    attaches several.  For every instruction carrying N>1 waits, keep one
    and hoist the rest onto EventSemaphore instructions inserted directly
    before it on the same engine (same block), which preserves each
    engine's program order and therefore the sync semantics.
    """
    import json as _json

    j = _json.loads(bir_json)
    uid = [0]
    for fn in j["functions"]:
        for blk in fn["blocks"]:
            out_insts = []
            for inst in blk["instructions"]:
                si = inst.get("sync_info")
                waits = (si or {}).get("on_wait") or []
                cap = 2 if inst.get("opcode") == "EventSemaphore" else 1
                if len(waits) > cap:
                    extra, keep = waits[:-cap], waits[-cap:]
                    for i in range(0, len(extra), 2):
                        uid[0] += 1
                        out_insts.append(
                            {
                                "name": f"antwaitfix-{uid[0]}",
                                "opcode": "EventSemaphore",
                                "engine": inst["engine"],
                                "ins": [],
                                "outs": [],
                                "debug": inst.get("debug", 0),
                                "sync_info": {
                                    "on_wait": extra[i : i + 2],
                                    "on_update": [],
                                },
                            }
                        )
                    si["on_wait"] = keep
                out_insts.append(inst)
            blk["instructions"] = out_insts
    return _json.dumps(j).encode()


def build_module():
    nc = bass.Bass(use_seq_codegen=True)

    xT = nc.declare_dram_parameter("xT", [D, L], BF16, isOutput=False)
    wq = nc.declare_dram_parameter("wq", [D, DQ], BF16, isOutput=False)
    wk = nc.declare_dram_parameter("wk", [D, DQ], BF16, isOutput=False)
    wv = nc.declare_dram_parameter("wv", [D, DQ], BF16, isOutput=False)
    wo = nc.declare_dram_parameter("wo", [DQ, D], BF16, isOutput=False)
    bq = nc.declare_dram_parameter("bq", [128, PAIRS], F32, isOutput=False)
    bk = nc.declare_dram_parameter("bk", [128, PAIRS], F32, isOutput=False)
    bv = nc.declare_dram_parameter("bv", [128, DQ], F32, isOutput=False)
    cosT = nc.declare_dram_parameter("cosT", [128, L], BF16, isOutput=False)
    sinT = nc.declare_dram_parameter("sinT", [128, L], BF16, isOutput=False)
    maskb = nc.declare_dram_parameter("maskb", [128, 896], BF16, isOutput=False)
    out = nc.declare_dram_parameter("out", [L, D], F32, isOutput=True)

    with tile.TileContext(nc) as tc:
        with (
            tc.tile_pool(name="const", bufs=1) as cp,
            tc.tile_pool(name="acts", bufs=1) as ap,
            tc.tile_pool(name="work", bufs=4) as wp,
            tc.tile_pool(name="sc", bufs=2, space="PSUM") as scp,
            tc.tile_pool(name="pv", bufs=2, space="PSUM") as pvp,
            tc.tile_pool(name="fp", bufs=2, space="PSUM") as fpp,
        ):
            # ---- input loads. The SP sequencer spends ~565ns configuring
            # each dma_start, so loads are issued as ONE config per tensor
            # (the descriptors still spread across DMA engines); the small
            # constants ride the ACT sequencer, which is idle until the
            # first exp. xT's first column-chunk is its own config so chunk-0
            # projection isn't gated on the full 4MB activation load.
            xT_sb = ap.tile([128, KT, L], BF16)
            wq_sb = cp.tile([128, KT, DQ], BF16)
            wk_sb = cp.tile([128, KT, DQ], BF16)
            wv_sb = cp.tile([128, KT, DQ], BF16)
            xTr = xT.rearrange("(kt p) l -> p kt l", p=128)
            nc.sync.dma_start(
                wq_sb[:], wq.rearrange("(kt p) m -> p kt m", p=128)
            )
            nc.sync.dma_start(xT_sb[:, :, 0:CHUNK], xTr[:, :, 0:CHUNK])
            nc.sync.dma_start(
                wk_sb[:], wk.rearrange("(kt p) m -> p kt m", p=128)
            )
            nc.sync.dma_start(
                wv_sb[:], wv.rearrange("(kt p) m -> p kt m", p=128)
            )
            bq_sb = cp.tile([128, PAIRS], F32)
            bk_sb = cp.tile([128, PAIRS], F32)
            bv_sb = cp.tile([128, DQ], F32)
            cos_sb = cp.tile([128, L], BF16)
            sin_sb = cp.tile([128, L], BF16)
            mask_sb = cp.tile([128, 896], BF16)
            nc.scalar.dma_start(bq_sb[:], bq[:])
            nc.scalar.dma_start(bk_sb[:], bk[:])
            nc.scalar.dma_start(cos_sb[:], cosT[:])
            nc.scalar.dma_start(sin_sb[:], sinT[:])
            nc.scalar.dma_start(bv_sb[:], bv[:])
            nc.scalar.dma_start(mask_sb[:], maskb[:])
            nc.sync.dma_start(
                xT_sb[:, :, CHUNK:L], xTr[:, :, CHUNK:L]
            )
            wo_sb = cp.tile([128, PAIRS, D], BF16)
            nc.sync.dma_start(
                wo_sb[:], wo.rearrange("(pr p) c -> p pr c", p=128)
            )

            # Selector rows for the denominator-broadcast matmuls:
            # sel[:, 0, :] = [1]*64 + [0]*64, sel[:, 1, :] = its complement.
            # memset can't encode a float32r immediate; memset f32 then
            # copy-convert (bitwise identical) into the f32r tile.
            sel_f32 = cp.tile([128, 2, 128], F32)
            nc.vector.memset(sel_f32[:, 0, 0:64], 1.0)
            nc.vector.memset(sel_f32[:, 0, 64:128], 0.0)
            nc.vector.memset(sel_f32[:, 1, 0:64], 0.0)
            nc.vector.memset(sel_f32[:, 1, 64:128], 1.0)
            sel_sb = cp.tile([128, 2, 128], mybir.dt.float32r)
            with nc.allow_low_precision(reason="f32r selectors for bcast mm"):
                nc.vector.tensor_copy(sel_sb[:], sel_f32[:])

            qT_sb = ap.tile([128, PAIRS, L], BF16)
            kT_sb = ap.tile([128, PAIRS, L], BF16)
            v_sb = ap.tile([128, LT, HPC * VW], BF16)
            yT_sb = ap.tile([128, PAIRS, L], BF16)
            # ones columns of V, set once for all l-tiles
            v4 = v_sb.rearrange("p lt (h c) -> p lt h c", c=VW)
            nc.vector.memset(v4[:, :, :, DH:VW], 1.0)

            def qk_group(c, mt, which):
                cs = slice(c * CHUNK, (c + 1) * CHUNK)
                dst, w_sb, b_sb = (
                    (qT_sb, wq_sb, bq_sb) if which == "q" else (kT_sb, wk_sb, bk_sb)
                )
                ps = fpp.tile(
                    [128, CHUNK], F32, tag="fp", name=f"{which}_{c}_{mt}"
                )
                for kt in range(KT):
                    nc.tensor.matmul(
                        ps[:],
                        w_sb[:, kt, mt * 128 : (mt + 1) * 128],
                        xT_sb[:, kt, cs],
                        start=(kt == 0),
                        stop=(kt == KT - 1),
                    )
                nc.vector.tensor_scalar_add(
                    dst[:, mt, cs], ps[:], b_sb[:, mt : mt + 1]
                )

            def v_group(lt):
                ps = fpp.tile([128, CHUNK], F32, tag="fp", name=f"v_{lt}")
                for kt in range(KT):
                    nc.tensor.matmul(
                        ps[:],
                        xT_sb[:, kt, lt * 128 : (lt + 1) * 128],
                        wv_sb[:, kt, :],
                        start=(kt == 0),
                        stop=(kt == KT - 1),
                    )
                vdst = v_sb[:, lt, :].rearrange("p (h c) -> p h c", c=VW)
                nc.vector.tensor_add(vdst[:, :, 0:DH], ps[:], bv_sb[:])

            def rope_group(c, mt):
                cs = slice(c * CHUNK, (c + 1) * CHUNK)
                for dst in (qT_sb, kT_sb):
                    t = dst[:, mt, cs]
                    swp = wp.tile([128, CHUNK], BF16, tag="swp",
                                  name=f"swp_{c}_{mt}")
                    for i in range(4):
                        j = i ^ 1
                        nc.sync.dma_start(
                            swp[i * 32 : (i + 1) * 32, :],
                            t[j * 32 : (j + 1) * 32, :],
                        )
                    nc.vector.tensor_mul(swp[:], swp[:], sin_sb[:, cs])
                    nc.vector.tensor_mul(t, t, cos_sb[:, cs])
                    nc.vector.tensor_add(t, t, swp[:])

            def proj_closures(c):
                fs = []
                for mt in range(PAIRS):
                    fs.append(lambda c=c, mt=mt: qk_group(c, mt, "q"))
                    fs.append(lambda c=c, mt=mt: qk_group(c, mt, "k"))
                    fs.append(lambda c=c, mt=mt: rope_group(c, mt))
                    if mt == 0:
                        for lt in range(4 * c, 4 * c + 4):
                            fs.append(lambda lt=lt: v_group(lt))
                return fs

            def outproj_group(lt, cc):
                ps = fpp.tile([128, CHUNK], F32, tag="fp", name=f"op_{lt}_{cc}")
                for pr in range(PAIRS):
                    nc.tensor.matmul(
                        ps[:],
                        yT_sb[:, pr, lt * 128 : (lt + 1) * 128],
                        wo_sb[:, pr, cc * CHUNK : (cc + 1) * CHUNK],
                        start=(pr == 0),
                        stop=(pr == PAIRS - 1),
                    )
                ob = wp.tile([128, CHUNK], F32, tag="ob", name=f"ob_{lt}_{cc}")
                nc.vector.tensor_copy(ob[:], ps[:])
                nc.sync.dma_start(
                    out[lt * 128 : (lt + 1) * 128, cc * CHUNK : (cc + 1) * CHUNK],
                    ob[:],
                )

            def outproj_closures(c):
                return [
                    lambda lt=lt, cc=cc: outproj_group(lt, cc)
                    for lt in range(4 * c, 4 * c + 4)
                    for cc in range(2)
                ]

            def attn_iter(c, pr, kt, ys, n_lk):
                q0 = c * CHUNK
                k0 = kt * 128
                sct = scp.tile(
                    [128, 2, CHUNK], F32, tag="sc", name=f"sc_{c}_{pr}_{kt}"
                )
                for hh in range(2):
                    nc.tensor.matmul(
                        sct[:, hh, :],
                        kT_sb[hh * 64 : (hh + 1) * 64, pr, k0 : k0 + 128],
                        qT_sb[hh * 64 : (hh + 1) * 64, pr, q0 : q0 + CHUNK],
                        start=True,
                        stop=True,
                    )
                ex = wp.tile(
                    [128, 2, CHUNK], BF16, tag="ex", name=f"ex_{c}_{pr}_{kt}"
                )
                nc.scalar.activation(
                    ex[:],
                    sct[:],
                    mybir.ActivationFunctionType.Exp,
                    scale=float(1.0 / np.sqrt(DH)),
                )
                if k0 >= q0:
                    s = 384 - (k0 - q0)
                    mbc = (
                        mask_sb[:, s : s + CHUNK]
                        .unsqueeze(1)
                        .broadcast_to([128, 2, CHUNK])
                    )
                    nc.vector.tensor_mul(ex[:], ex[:], mbc)
                for hh in range(2):
                    h = 2 * pr + hh
                    nc.tensor.matmul(
                        ys[hh][0:VW, :],
                        v_sb[:, kt, h * VW : (h + 1) * VW],
                        ex[:, hh, :],
                        start=(kt == 0),
                        stop=(kt == n_lk - 1),
                    )

            def attn_epilogue(c, pr, ys):
                # normalize by the denominator row (row 64). PSUM can't feed
                # DMA or matmul-rhs, so: DVE copies the raw denominator rows
                # PSUM->SBUF with f32r rounding (the verifier demands an
                # f32r-rounding producer), two accumulating selector-matmuls
                # broadcast them (hh0 -> psum rows 0:64, hh1 -> 64:128), and
                # ACT computes 1/x as exp(-ln(x)) while staging to SBUF (both
                # funcs share one table set; DVE reciprocal costs 6.5ns/elem
                # and the custom-DVE approx ops don't survive this walrus).
                # A SBUF->SBUF DMA moves the hh1 half down to base 0
                # (elementwise engines cannot change partition base).
                q0 = c * CHUNK
                den_r = wp.tile(
                    [128, 2, CHUNK], mybir.dt.float32r, tag="denr", bufs=2,
                    name=f"denr_{c}_{pr}",
                )
                with nc.allow_low_precision(reason="f32r denom rounding"):
                    for hh in range(2):
                        nc.vector.tensor_copy(
                            den_r[64:65, hh, :], ys[hh][64:65, :]
                        )
                bc_ps = fpp.tile([128, CHUNK], F32, tag="fp", name=f"bc_{c}_{pr}")
                for hh in range(2):
                    nc.tensor.matmul(
                        bc_ps[:],
                        sel_sb[64:65, hh, :],
                        den_r[64:65, hh, :],
                        start=(hh == 0),
                        stop=(hh == 1),
                    )
                lnb = wp.tile([128, CHUNK], F32, tag="lnb", bufs=2,
                              name=f"lnb_{c}_{pr}")
                nc.scalar.activation(
                    lnb[:], bc_ps[:], mybir.ActivationFunctionType.Ln
                )
                bcs = wp.tile([128, CHUNK], F32, tag="bcs", bufs=2,
                              name=f"bcs_{c}_{pr}")
                nc.scalar.activation(
                    bcs[:], lnb[:], mybir.ActivationFunctionType.Exp,
                    scale=-1.0,
                )
                bcs1 = wp.tile([64, CHUNK], F32, tag="bcs1", bufs=2,
                               name=f"bcs1_{c}_{pr}")
                nc.sync.dma_start(bcs1[:], bcs[64:128, :])
                nc.vector.tensor_mul(
                    yT_sb[0:64, pr, q0 : q0 + CHUNK], ys[0][0:64, :], bcs[0:64, :]
                )
                yt = wp.tile([64, CHUNK], BF16, tag="yt", name=f"yt_{c}_{pr}")
                nc.vector.tensor_mul(yt[:], ys[1][0:64, :], bcs1[:])
                nc.sync.dma_start(yT_sb[64:128, pr, q0 : q0 + CHUNK], yt[:])

            def attn_chunk(c, fillers):
                """Emit chunk c's attention iterations with filler groups
                interleaved evenly (priority-spreading: the list scheduler
                prefers earlier-emitted work, so clustering fillers starves
                ACT of exp work while PE grinds through them)."""
                n_lk = 4 * (c + 1)
                n_iters = PAIRS * n_lk
                pace = len(fillers) / max(n_iters, 1)
                credit = 0.0
                for pr in range(PAIRS):
                    ys = [
                        pvp.tile(
                            [128, CHUNK], F32, tag="pv", name=f"ys_{c}_{pr}_{hh}"
                        )
                        for hh in range(2)
                    ]
                    for kt in range(n_lk):
                        attn_iter(c, pr, kt, ys, n_lk)
                        credit += pace
                        while credit >= 1.0 and fillers:
                            fillers.popleft()()
                            credit -= 1.0
                    attn_epilogue(c, pr, ys)
                while fillers:
                    fillers.popleft()()

            from collections import deque

            p0 = proj_closures(0)
            # startup: emit only what attention(0, pr=0) needs (q0/k0/rope0 +
            # the four v tiles), everything else becomes interleaved filler.
            for f in p0[:7]:
                f()
            attn_chunk(0, deque(p0[7:] + proj_closures(1)))
            attn_chunk(1, deque(proj_closures(2) + outproj_closures(0)))
            attn_chunk(2, deque(proj_closures(3) + outproj_closures(1)))
            attn_chunk(3, deque(outproj_closures(2)))
            for f in outproj_closures(3):
                f()
    return nc


def _rope_tables():
    inv_freq = (1.0 / (10000.0 ** (np.arange(0, DH, 2, dtype=np.float32) / DH))).astype(
        np.float32
    )
    t = np.arange(L, dtype=np.float32)
    freqs = np.einsum("l,d->ld", t, inv_freq).astype(np.float32)  # (L, 32)
    emb = np.concatenate([freqs, freqs], axis=-1)                 # (L, 64)
    cos = np.cos(emb).astype(np.float32)
    sin = np.sin(emb).astype(np.float32)
    cosT = cos.T                                   # (64, L)
    sinT = sin.T.copy()
    sinT[0:32] = -sinT[0:32]                       # fold rotate_half sign
    cos128 = np.tile(cosT, (2, 1))                 # (128, L)
    sin128 = np.tile(sinT, (2, 1))
    return cos128, sin128


def _mask_big():
    # maskb[p, j] = 1.0 iff p <= j - 384 (slice at s = 384-delta gives the
    # diagonal-tile mask "p <= f - delta")
    p = np.arange(128)[:, None]
    j = np.arange(896)[None, :]
    return (p <= j - 384).astype(np.float32)


def _bf16(a):
    return np.asarray(a, dtype=np.float32).astype(ml_dtypes.bfloat16)


_COMPILED = None


def _ensure_trace_hook() -> bool:
    """Install the axon NTFF profile hook if the boot shim couldn't.

    The image's `antenv` stub lacks `axon_hooks`, so bass_utils' trace
    path crashes on import. Synthesize the module and wire in the ctypes
    hook from trn_agent_boot. Returns True iff tracing is usable.
    """
    try:
        from antenv.axon_hooks import get_axon_ntff_profile_hook  # noqa: F401

        return True
    except ImportError:
        pass
    try:
        import types

        import antenv
        import trn_agent_boot.trn_boot as tb

        mod = types.ModuleType("antenv.axon_hooks")
        _hook = [None]
        mod.set_axon_ntff_profile_hook = lambda h: _hook.__setitem__(0, h)
        mod.get_axon_ntff_profile_hook = lambda: _hook[0]
        sys.modules["antenv.axon_hooks"] = mod
        antenv.axon_hooks = mod
        mod.set_axon_ntff_profile_hook(
            tb._ntff_profile_via_ctypes("/opt/axon/libaxon_pjrt.so")
        )
        return True
    except Exception:
        return False


def kernel(x, pad_mask, W_qkv, b_qkv, W_out, b_out):
    global LAST_RESULTS, _COMPILED
    from concourse.bass_utils import run_bass_kernel_spmd

    x = np.asarray(x, dtype=np.float32)
    W_qkv = np.asarray(W_qkv, dtype=np.float32)
    b_qkv = np.asarray(b_qkv, dtype=np.float32)
    W_out = np.asarray(W_out, dtype=np.float32)
    b_out = np.asarray(b_out, dtype=np.float32)

    cos128, sin128 = _rope_tables()
    maskb = _mask_big()

    in_maps = []
    for core in range(NCORES):
        b, g = core // G, core % G
        sl = slice(g * DQ, (g + 1) * DQ)
        wq = W_qkv[:, 0 * D : 1 * D][:, sl]
        wk = W_qkv[:, 1 * D : 2 * D][:, sl]
        wv = W_qkv[:, 2 * D : 3 * D][:, sl]
        bqv = b_qkv[0 * D : 1 * D][sl]
        bkv = b_qkv[1 * D : 2 * D][sl]
        bvv = b_qkv[2 * D : 3 * D][sl]
        in_maps.append(
            {
                "xT": _bf16(x[b].T),
                "wq": _bf16(wq),
                "wk": _bf16(wk),
                "wv": _bf16(wv),
                "wo": _bf16(W_out[sl, :]),
                "bq": np.ascontiguousarray(bqv.reshape(PAIRS, 128).T),
                "bk": np.ascontiguousarray(bkv.reshape(PAIRS, 128).T),
                "bv": np.tile(bvv[None, :], (128, 1)).astype(np.float32),
                "cosT": _bf16(cos128),
                "sinT": _bf16(sin128),
                "maskb": _bf16(maskb),
            }
        )

    if _COMPILED is None:
        nc = build_module()
        fixed = legalize_bir_waits(nc.to_json_bytes())
        nc.to_json_bytes = lambda: fixed  # bass2jax ships this BIR to walrus
        _COMPILED = nc
    nc = _COMPILED

    res = run_bass_kernel_spmd(
        nc,
        in_maps,
        core_ids=list(range(NCORES)),
        trace=bool(os.environ.get("BASS_TRACE")) and _ensure_trace_hook(),
    )
    LAST_RESULTS = res

    out = np.zeros((B, L, D), dtype=np.float32)
    for core in range(NCORES):
        out[core // G] += np.asarray(res.results[core]["out"], dtype=np.float32)
    out += b_out[None, None, :]
    return out
